# revision 1
# baseline (speedup 1.0000x reference)
"""MLA-style attention (DeepSeek MLA block) on 8 Trainium2 NeuronCores.

Sharding: core c = b*4 + g  (batch b in {0,1}, head-group g in {0..3} = 4 heads).
Each core computes its batch's full low-rank projections (replicated across the
4 head-group cores of that batch), its 4 heads' q/k/v + causal attention, and a
partial output projection; the host sums the 4 partials per batch.

Device layout is feature-major ("transposed") end-to-end: activations live as
(features, tokens) so every matmul contraction dim is the partition dim and no
on-chip transposes are needed. RMSNorm weight vectors are folded into the
following weight matrix on the host; the per-token rsqrt scale is applied after
the matmul (it commutes). Rope pairs are de-interleaved to [reals | imags] via
host-side row permutation of wq_b / wkv_a. Softmax skips max-subtraction
(scores are O(3.5) for this model family) and the row-sum comes from a
ones-vector matmul. All matmuls run as float32r (fp32 data, ~fp22 multiply,
fp32 accumulate).
"""
import numpy as np

import concourse.bass as bass
import concourse.tile as tile
from concourse import bacc, mybir
from concourse.bass_utils import run_bass_kernel_spmd

F32 = mybir.dt.float32
F32R = mybir.dt.float32r

B, S, DIM = 2, 2048, 2048
NH = 16
QL, KVL = 1536, 512
NOPE, ROPE, VHD = 128, 64, 128
QK_HD = NOPE + ROPE
EPS = 1e-6
SCALE = QK_HD ** -0.5
HG = 4            # heads per group
T = S             # tokens per core (one batch)
NEG = -1e30
P = 128
NCH = T // 512    # 512-token chunks
KT_D = DIM // P   # 16 contraction tiles over model dim
MT_Q = QL // P    # 12 q_lora tiles
MT_QB = 768 // P  # 6 output tiles of sliced wq_b
KT_QL = QL // P   # 12
KT_KV = KVL // P  # 4
TT = T // P       # 16 token tiles


def r32(ap):
    return ap.bitcast(F32R)


# ---------------------------------------------------------------- host side

def _host_prep(inp):
    cos = np.asarray(inp["freqs_cos"], np.float32)
    sin = np.asarray(inp["freqs_sin"], np.float32)
    cosT4 = np.ascontiguousarray(np.tile(cos.T, (4, 1)))  # (128, S)
    sinT4 = np.ascontiguousarray(np.tile(sin.T, (4, 1)))

    wqaT = np.ascontiguousarray(np.asarray(inp["wq_a_w"], np.float32).T)

    perm_kva = np.concatenate([
        np.arange(KVL),
        KVL + 2 * np.arange(32),
        KVL + 2 * np.arange(32) + 1,
    ])
    wkvaT = np.ascontiguousarray(np.asarray(inp["wkv_a_w"], np.float32)[perm_kva].T)

    qn = np.asarray(inp["q_norm_w"], np.float32)
    wqb = np.asarray(inp["wq_b_w"], np.float32) * qn[None, :]
    bqb = np.asarray(inp["wq_b_b"], np.float32)
    kvn = np.asarray(inp["kv_norm_w"], np.float32)
    wkvb = np.asarray(inp["wkv_b_w"], np.float32) * kvn[None, :]
    bkvb = np.asarray(inp["wkv_b_b"], np.float32)
    wo = np.asarray(inp["wo_w"], np.float32)

    masks = np.zeros((4, P, 512), np.float32)
    for di, delta in enumerate((0, 128, 256, 384)):
        kk = np.arange(P)[:, None] + delta
        qq = np.arange(512)[None, :]
        masks[di] = np.where(kk > qq, NEG, 0.0)

    bqa = np.asarray(inp["wq_a_b"], np.float32).reshape(MT_Q, P)
    bkva = np.zeros((5, P), np.float32)
    bkva.reshape(-1)[:576] = np.asarray(inp["wkv_a_b"], np.float32)[perm_kva]

    cores = []
    for b in range(B):
        xt = np.ascontiguousarray(np.asarray(inp["x"], np.float32)[b].T)
        for g in range(HG):
            heads = range(4 * g, 4 * g + 4)
            rows_nope = np.concatenate([np.arange(h * QK_HD, h * QK_HD + NOPE) for h in heads])
            rows_real = np.concatenate([h * QK_HD + NOPE + 2 * np.arange(32) for h in heads])
            rows_imag = np.concatenate([h * QK_HD + NOPE + 2 * np.arange(32) + 1 for h in heads])
            rows_q = np.concatenate([rows_nope, rows_real, rows_imag])
            rows_k = np.concatenate([np.arange(h * (NOPE + VHD), h * (NOPE + VHD) + NOPE) for h in heads])
            rows_v = np.concatenate([np.arange(h * (NOPE + VHD) + NOPE, (h + 1) * (NOPE + VHD)) for h in heads])
            # packed per-feature scalars: cols 0:12 bqa | 12:17 bkva | 17:23
            # bqb | 23:27 bk | 27:31 bv | 31 eps
            biases = np.zeros((32, P), np.float32)
            biases[0:12] = bqa
            biases[12:17] = bkva
            biases[17:23] = bqb[rows_q].reshape(MT_QB, P)
            biases[23:27] = bkvb[rows_k].reshape(4, P)
            biases[27:31] = bkvb[rows_v].reshape(4, P)
            biases[31] = EPS
            cores.append(dict(
                xt=xt,
                wqaT=wqaT,
                wkvaT=wkvaT,
                wqbT=np.ascontiguousarray(wqb[rows_q].T),
                wkvbTk=np.ascontiguousarray(wkvb[rows_k].T),
                wkvbTv=np.ascontiguousarray(wkvb[rows_v].T),
                woT=np.ascontiguousarray(wo[:, 512 * g: 512 * (g + 1)].T),
                biases=biases,
                cosT4=cosT4, sinT4=sinT4, masks=masks,
                ones_in=np.ones((P, 1), np.float32),
            ))
    return cores


INPUT_SPECS = dict(
    xt=(DIM, T), wqaT=(DIM, QL),
    wkvaT=(DIM, 576),
    wqbT=(QL, 768),
    wkvbTk=(KVL, 512),
    wkvbTv=(KVL, 512),
    woT=(512, DIM), ones_in=(P, 1),
    biases=(32, P),
    cosT4=(P, T), sinT4=(P, T), masks=(4, P, 512),
)


# ---------------------------------------------------------------- device IR

def _col_block(w, m, width=P):
    """AP over DRAM weight w (R, C): (128p over rows, R//128 ktiles, width cols
    starting at m*128)."""
    rows, cols = w.shape
    return bass.AP(
        tensor=w.tensor, offset=m * P,
        ap=[[cols, P], [P * cols, rows // P], [1, width]],
    )


def _t_view(a2d):
    """AP over host (A, P)-shaped DRAM tensor as (P partitions, A)."""
    arows, acols = a2d.shape
    assert acols == P
    return bass.AP(tensor=a2d.tensor, offset=0, ap=[[1, P], [P, arows]])


def build_bass():
    nc = bacc.Bacc("TRN2", target_bir_lowering=False, debug=False, num_devices=8)

    R_IN = {"xt", "wqaT", "wqbT", "wkvaT", "wkvbTk", "wkvbTv", "woT", "ones_in"}
    din = {name: nc.dram_tensor(name, shape, F32R if name in R_IN else F32,
                                kind="ExternalInput").ap()
           for name, shape in INPUT_SPECS.items()}
    outT = nc.dram_tensor("outT", (DIM, T), F32, kind="ExternalOutput").ap()
    scratch = dict(
        qmid_d=nc.dram_tensor("qmid_d", (QL, T), F32R).ap(),
        kvc_d=nc.dram_tensor("kvc_d", (KVL, T), F32R).ap(),
        qT_d=nc.dram_tensor("qT_d", (768, T), F32R).ap(),
        knope_d=nc.dram_tensor("knope_d", (512, T), F32R).ap(),
        v_d=nc.dram_tensor("v_d", (T, 512), F32R).ap(),
        o_d=nc.dram_tensor("o_d", (512, T), F32R).ap(),
    )

    with tile.TileContext(nc) as tc:
        _emit(tc, din, outT, scratch)

    nc.compile()
    return nc


def _emit(tc, din, outT, scratch):
    nc = tc.nc
    from contextlib import ExitStack
    ALU = mybir.AluOpType
    AF = mybir.ActivationFunctionType
    qmid_d, kvc_d, qT_d, knope_d, v_d, o_d = (
        scratch["qmid_d"], scratch["kvc_d"], scratch["qT_d"],
        scratch["knope_d"], scratch["v_d"], scratch["o_d"])

    with ExitStack() as outer:
        const = outer.enter_context(tc.tile_pool(name="const", bufs=1))
        ones = const.tile([P, 1], F32R)
        nc.sync.dma_start(out=ones, in_=din["ones_in"])
        kpeT = const.tile([64, T], F32R)  # roped shared k_pe, [real|imag] rows
        bs = const.tile([P, 32], F32)     # packed per-feature scalars
        nc.sync.dma_start(out=bs, in_=_t_view(din["biases"]))
        bqa_sb, bkva_sb = bs[:, 0:12], bs[:, 12:17]
        bqb_sb, bk_sb, bv_sb = bs[:, 17:23], bs[:, 23:27], bs[:, 27:31]
        eps_sb = bs[:, 31:32]

        with ExitStack() as stats_scope:
            spool = stats_scope.enter_context(tc.tile_pool(name="stats", bufs=1))
            ssq_q = spool.tile([1, T], F32)
            ssq_kv = spool.tile([1, T], F32)
            rq_row = spool.tile([1, T], F32)
            rkv_row = spool.tile([1, T], F32)
            rkv_tok = spool.tile([P, TT], F32)

            # ------------ P1: q_mid^T, kv^T from x^T (two token-halves) ----
            HT = T // 2
            HNCH = HT // 512
            for th in range(2):
                t0 = th * HT
                with tc.tile_pool(name="p1x", bufs=1) as xpool, \
                     tc.tile_pool(name="p1", bufs=4) as wpool, \
                     tc.tile_pool(name="p1c", bufs=3) as cpool, \
                     tc.tile_pool(name="p1ps", bufs=6, space="PSUM") as pspool, \
                     tc.tile_pool(name="p1se", bufs=2, space="PSUM") as sepool:
                    # first q weight block ahead of the bulk x load so the
                    # first matmul isn't queued behind 8.4MB of DMA
                    wcb0 = wpool.tile([P, KT_D, P], F32R, tag="wcb", name="wcb0")
                    nc.sync.dma_start(out=wcb0, in_=_col_block(din["wqaT"], 0))
                    x_sb = []
                    for k in range(KT_D):
                        xk = xpool.tile([P, HT], F32R, tag=f"x{k}", name=f"x{k}")
                        nc.sync.dma_start(out=xk, in_=din["xt"][P * k:P * (k + 1),
                                                                t0:t0 + HT])
                        x_sb.append(xk)
                    cosT_sb = xpool.tile([32, HT], F32)
                    nc.sync.dma_start(out=cosT_sb,
                                      in_=din["cosT4"][0:32, t0:t0 + HT])
                    sinT_sb = xpool.tile([32, HT], F32)
                    nc.sync.dma_start(out=sinT_sb,
                                      in_=din["sinT4"][0:32, t0:t0 + HT])

                    for part in ("q", "kv"):
                        n_m = MT_Q if part == "q" else 5
                        w_d = din["wqaT"] if part == "q" else din["wkvaT"]
                        ses = [sepool.tile([1, 512], F32, tag="se",
                                           name=f"se{part}{nn}")
                               for nn in range(HNCH)]
                        n_acc = MT_Q if part == "q" else 4
                        for m in range(n_m):
                            width = P if (part == "q" or m < 4) else 64
                            if part == "q" and m == 0:
                                wcb = wcb0
                            else:
                                wcb = wpool.tile([P, KT_D, P], F32R, tag="wcb",
                                                 name="wcb")
                                nc.sync.dma_start(out=wcb[:, :, :width],
                                                  in_=_col_block(w_d, m, width))
                            pss = [pspool.tile([P, 512], F32, tag="mm",
                                               name=f"ps{nn}")
                                   for nn in range(HNCH)]
                            # consecutive same-bank accumulation chains: HW
                            # probe shows 271ns/MM vs 355 for bank-alternating
                            for nn in range(HNCH):
                                for k in range(KT_D):
                                    nc.tensor.matmul(
                                        pss[nn][:width, :], r32(wcb[:, k, :width]),
                                        r32(x_sb[k][:, 512 * nn:512 * (nn + 1)]),
                                        start=(k == 0), stop=(k == KT_D - 1))
                            for nn in range(HNCH):
                                sl = slice(512 * nn, 512 * (nn + 1))
                                gsl = slice(t0 + 512 * nn, t0 + 512 * (nn + 1))
                                ps = pss[nn]
                                if part == "q" or m < 4:
                                    bias = (bqa_sb[:, m:m + 1] if part == "q"
                                            else bkva_sb[:, m:m + 1])
                                    dst = (qmid_d if part == "q" else kvc_d)
                                    ch = cpool.tile([P, 512], F32R, tag="ch",
                                                    name="ch")
                                    nc.vector.tensor_scalar_add(ch, ps, bias)
                                    nc.sync.dma_start(
                                        out=dst[P * m:P * (m + 1), gsl], in_=ch)
                                    sq = cpool.tile([P, 512], F32R, tag="sq",
                                                    name="sq")
                                    nc.vector.tensor_tensor(sq, ch, ch, ALU.mult)
                                    nc.tensor.matmul(ses[nn], r32(ones), r32(sq),
                                                     start=(m == 0),
                                                     stop=(m == n_acc - 1))
                                else:
                                    ch = cpool.tile([P, 512], F32R, tag="ch",
                                                    name="ch3")
                                    nc.vector.tensor_scalar_add(
                                        ch[:64, :], ps[:64, :], bkva_sb[0:64, 4:5])
                                    # rope rotate k_pe: shift imag half down via
                                    # SBUF-SBUF DMA, rotate on partitions 0:32
                                    xi = cpool.tile([32, 512], F32R, tag="xi",
                                                    name="xi", bufs=2)
                                    nc.sync.dma_start(out=xi, in_=ch[32:64, :])
                                    xr = ch[0:32, :]
                                    t1 = cpool.tile([32, 512], F32, tag="t1",
                                                    name="t1", bufs=2)
                                    t2 = cpool.tile([32, 512], F32, tag="t2",
                                                    name="t2", bufs=2)
                                    yi = cpool.tile([32, 512], F32R, tag="yikp",
                                                    name="yikp", bufs=2)
                                    c_, s_ = cosT_sb[:, sl], sinT_sb[:, sl]
                                    nc.vector.tensor_tensor(t1, xr, c_, ALU.mult)
                                    nc.vector.tensor_tensor(t2, xi, s_, ALU.mult)
                                    nc.vector.tensor_tensor(kpeT[0:32, gsl], t1,
                                                            t2, ALU.subtract)
                                    nc.vector.tensor_tensor(t1, xr, s_, ALU.mult)
                                    nc.vector.tensor_tensor(t2, xi, c_, ALU.mult)
                                    nc.vector.tensor_tensor(yi, t1, t2, ALU.add)
                                    nc.sync.dma_start(out=kpeT[32:64, gsl], in_=yi)
                        # drain sumsq psums into the global rows
                        acc = ssq_q if part == "q" else ssq_kv
                        for nn in range(HNCH):
                            gsl = slice(t0 + 512 * nn, t0 + 512 * (nn + 1))
                            nc.vector.tensor_copy(acc[:, gsl], ses[nn])

            # finalize rms rows: r = 1/sqrt(mean + eps)
            srt_q = spool.tile([1, T], F32)
            nc.scalar.activation(srt_q, ssq_q, AF.Sqrt, bias=eps_sb[0:1, :],
                                 scale=1.0 / QL)
            nc.vector.reciprocal(rq_row, srt_q)
            srt_kv = spool.tile([1, T], F32)
            nc.scalar.activation(srt_kv, ssq_kv, AF.Sqrt, bias=eps_sb[0:1, :],
                                 scale=1.0 / KVL)
            nc.vector.reciprocal(rkv_row, srt_kv)
            # token-major copy of rkv for the v eviction
            for tt in range(TT):
                nc.sync.dma_start(out=rkv_tok[:, tt:tt + 1],
                                  in_=rkv_row[:, P * tt:P * (tt + 1)])

            # ------------ P2a: q^T = wqb^T-slice @ q_mid^T ----------------
            for half in range(2):
                h0 = half * HT
                with tc.tile_pool(name="p2a", bufs=1) as apool, \
                     tc.tile_pool(name="p2aw", bufs=3) as wpool, \
                     tc.tile_pool(name="p2ac", bufs=3) as cpool, \
                     tc.tile_pool(name="p2aps", bufs=4, space="PSUM") as pspool:
                    wq0 = wpool.tile([P, KT_QL, P], F32R, tag="wq", name="wq0")
                    nc.sync.dma_start(out=wq0, in_=_col_block(din["wqbT"], 0))
                    qm_sb = []
                    for k in range(KT_QL):
                        qmk = apool.tile([P, HT], F32R, tag=f"qm{k}", name=f"qm{k}")
                        nc.sync.dma_start(out=qmk,
                                          in_=qmid_d[P * k:P * (k + 1), h0:h0 + HT])
                        qm_sb.append(qmk)
                    rq_bc = apool.tile([P, HT], F32)
                    nc.gpsimd.partition_broadcast(rq_bc, rq_row[:, h0:h0 + HT])

                    for m in range(4):
                        if m == 0:
                            wcb = wq0
                        else:
                            wcb = wpool.tile([P, KT_QL, P], F32R, tag="wq",
                                             name="wq")
                            nc.sync.dma_start(out=wcb, in_=_col_block(din["wqbT"], m))
                        pss = [pspool.tile([P, 512], F32, tag="mm",
                                           name=f"psq{nn}") for nn in range(HNCH)]
                        for nn in range(HNCH):
                            for k in range(KT_QL):
                                nc.tensor.matmul(
                                    pss[nn], r32(wcb[:, k, :]),
                                    r32(qm_sb[k][:, 512 * nn:512 * (nn + 1)]),
                                    start=(k == 0), stop=(k == KT_QL - 1))
                        for nn in range(HNCH):
                            sl = slice(512 * nn, 512 * (nn + 1))
                            gsl = slice(h0 + 512 * nn, h0 + 512 * (nn + 1))
                            ch = cpool.tile([P, 512], F32R, tag="ch", name="chq")
                            nc.vector.tensor_tensor(ch, pss[nn], rq_bc[:, sl],
                                                    ALU.mult)
                            nc.vector.tensor_scalar_add(ch, ch, bqb_sb[:, m:m + 1])
                            nc.sync.dma_start(out=qT_d[P * m:P * (m + 1), gsl],
                                              in_=ch)

                    # rope tiles m=4 (reals), m=5 (imags)
                    wcb4 = wpool.tile([P, KT_QL, P], F32R, tag="wq", name="wq4")
                    nc.sync.dma_start(out=wcb4, in_=_col_block(din["wqbT"], 4))
                    wcb5 = wpool.tile([P, KT_QL, P], F32R, tag="wq", name="wq5")
                    nc.sync.dma_start(out=wcb5, in_=_col_block(din["wqbT"], 5))
                    for nn in range(HNCH):
                        sl = slice(512 * nn, 512 * (nn + 1))
                        gsl = slice(h0 + 512 * nn, h0 + 512 * (nn + 1))
                        c_ = cpool.tile([P, 512], F32, tag="c4", name="c4", bufs=2)
                        nc.sync.dma_start(out=c_, in_=din["cosT4"][:, gsl])
                        s_ = cpool.tile([P, 512], F32, tag="s4", name="s4", bufs=2)
                        nc.sync.dma_start(out=s_, in_=din["sinT4"][:, gsl])
                        chs = []
                        for mi, (m, wcb_) in enumerate(((4, wcb4), (5, wcb5))):
                            ps = pspool.tile([P, 512], F32, tag="mm", name="psr")
                            for k in range(KT_QL):
                                nc.tensor.matmul(ps, r32(wcb_[:, k, :]),
                                                 r32(qm_sb[k][:, sl]),
                                                 start=(k == 0),
                                                 stop=(k == KT_QL - 1))
                            ch = cpool.tile([P, 512], F32, tag=f"chr{mi}",
                                            name=f"chr{mi}", bufs=2)
                            nc.vector.tensor_tensor(ch, ps, rq_bc[:, sl], ALU.mult)
                            nc.vector.tensor_scalar_add(ch, ch, bqb_sb[:, m:m + 1])
                            chs.append(ch)
                        xr, xi = chs
                        t1 = cpool.tile([P, 512], F32, tag="t1", name="t1r", bufs=2)
                        t2 = cpool.tile([P, 512], F32, tag="t2", name="t2r", bufs=2)
                        yr = cpool.tile([P, 512], F32R, tag="yr", name="yr", bufs=2)
                        yi = cpool.tile([P, 512], F32R, tag="yi", name="yi", bufs=2)
                        nc.vector.tensor_tensor(t1, xr, c_, ALU.mult)
                        nc.vector.tensor_tensor(t2, xi, s_, ALU.mult)
                        nc.vector.tensor_tensor(yr, t1, t2, ALU.subtract)
                        nc.vector.tensor_tensor(t1, xr, s_, ALU.mult)
                        nc.vector.tensor_tensor(t2, xi, c_, ALU.mult)
                        nc.vector.tensor_tensor(yi, t1, t2, ALU.add)
                        nc.sync.dma_start(out=qT_d[512:640, gsl], in_=yr)
                        nc.sync.dma_start(out=qT_d[640:768, gsl], in_=yi)

            # ------------ P2b: k_nope^T and v from kv_c^T ----------------
            with tc.tile_pool(name="p2b", bufs=1) as bpool, \
                 tc.tile_pool(name="p2bc", bufs=3) as cpool, \
                 tc.tile_pool(name="p2bps", bufs=8, space="PSUM") as pspool:
                kvc_sb, wk_sb, wv_sb = [], [], []
                for k in range(KT_KV):
                    kvk = bpool.tile([P, T], F32R, tag=f"kvc{k}", name=f"kvc{k}")
                    nc.sync.dma_start(out=kvk, in_=kvc_d[P * k:P * (k + 1), :])
                    kvc_sb.append(kvk)
                    wkk = bpool.tile([P, 512], F32R, tag=f"wkk{k}", name=f"wkk{k}")
                    nc.sync.dma_start(out=wkk,
                                      in_=din["wkvbTk"][P * k:P * (k + 1), :])
                    wk_sb.append(wkk)
                    wvk = bpool.tile([P, 512], F32R, tag=f"wvk{k}", name=f"wvk{k}")
                    nc.sync.dma_start(out=wvk,
                                      in_=din["wkvbTv"][P * k:P * (k + 1), :])
                    wv_sb.append(wvk)
                rkv_bc = bpool.tile([P, T], F32)
                nc.gpsimd.partition_broadcast(rkv_bc, rkv_row)

                for m in range(4):
                    pss = [pspool.tile([P, 512], F32, tag="mm", name=f"psk{nn}")
                           for nn in range(NCH)]
                    for nn in range(NCH):
                        for k in range(KT_KV):
                            nc.tensor.matmul(
                                pss[nn], r32(wk_sb[k][:, P * m:P * (m + 1)]),
                                r32(kvc_sb[k][:, 512 * nn:512 * (nn + 1)]),
                                start=(k == 0), stop=(k == KT_KV - 1))
                    for nn in range(NCH):
                        sl = slice(512 * nn, 512 * (nn + 1))
                        ch = cpool.tile([P, 512], F32R, tag="ch", name="chk")
                        nc.vector.tensor_tensor(ch, pss[nn], rkv_bc[:, sl],
                                                ALU.mult)
                        nc.vector.tensor_scalar_add(ch, ch, bk_sb[:, m:m + 1])
                        nc.sync.dma_start(out=knope_d[P * m:P * (m + 1), sl],
                                          in_=ch)

                for tt in range(TT):
                    ps = pspool.tile([P, 512], F32, tag="mm", name="psv")
                    for k in range(KT_KV):
                        nc.tensor.matmul(ps,
                                         r32(kvc_sb[k][:, P * tt:P * (tt + 1)]),
                                         r32(wv_sb[k]), start=(k == 0),
                                         stop=(k == KT_KV - 1))
                    ch = cpool.tile([P, 512], F32R, tag="ch", name="chv")
                    nc.vector.tensor_scalar(ch, ps, rkv_tok[:, tt:tt + 1], None,
                                            ALU.mult)
                    nc.sync.dma_start(out=v_d[P * tt:P * (tt + 1), :], in_=ch)

        # ------------ P3: causal attention per head ----------------
        with tc.tile_pool(name="p3", bufs=1) as hpool, \
             tc.tile_pool(name="p3h", bufs=2) as h2pool, \
             tc.tile_pool(name="p3c", bufs=2) as cpool, \
             tc.tile_pool(name="p3e", bufs=2) as epool, \
             tc.tile_pool(name="p3ps", bufs=4, space="PSUM") as pspool, \
             tc.tile_pool(name="p3o", bufs=2, space="PSUM") as opool, \
             tc.tile_pool(name="p3se", bufs=2, space="PSUM") as sepool:
            masks_sb = hpool.tile([P, 4, 512], F32)
            nc.sync.dma_start(
                out=masks_sb,
                in_=bass.AP(tensor=din["masks"].tensor, offset=0,
                            ap=[[512, P], [P * 512, 4], [1, 512]]))

            for h in range(HG):
                qn = h2pool.tile([P, T], F32R, tag="qn", name="qn")
                nc.sync.dma_start(out=qn, in_=qT_d[P * h:P * (h + 1), :])
                qr = h2pool.tile([64, T], F32R, tag="qr", name="qr")
                nc.sync.dma_start(out=qr[0:32, :],
                                  in_=qT_d[512 + 32 * h:544 + 32 * h, :])
                nc.sync.dma_start(out=qr[32:64, :],
                                  in_=qT_d[640 + 32 * h:672 + 32 * h, :])
                kn = h2pool.tile([P, T], F32R, tag="kn", name="kn")
                nc.sync.dma_start(out=kn, in_=knope_d[P * h:P * (h + 1), :])
                vh = h2pool.tile([P, TT, P], F32R, tag="vh", name="vh")
                nc.sync.dma_start(
                    out=vh,
                    in_=bass.AP(tensor=v_d.tensor, offset=P * h,
                                ap=[[512, P], [P * 512, TT], [1, P]]))

                for qch in range(NCH):
                    qsl = slice(512 * qch, 512 * (qch + 1))
                    n_kt = 4 * (qch + 1)
                    es = epool.tile([P, TT, 512], F32R, tag="es", name="es")
                    for kt in range(n_kt):
                        ps = pspool.tile([P, 512], F32, tag="s", name="s")
                        nc.tensor.matmul(ps, r32(kn[:, P * kt:P * (kt + 1)]),
                                         r32(qn[:, qsl]), start=True, stop=False)
                        nc.tensor.matmul(ps, r32(kpeT[:, P * kt:P * (kt + 1)]),
                                         r32(qr[:, qsl]), start=False, stop=True)
                        di = kt - 4 * qch
                        if di >= 0:
                            nc.vector.tensor_tensor(ps, ps, masks_sb[:, di, :],
                                                    ALU.add)
                        nc.scalar.activation(es[:, kt, :], ps, AF.Exp,
                                             scale=SCALE)
                    o_ps = opool.tile([P, 512], F32, tag="o", name="o")
                    for kt in range(n_kt):
                        nc.tensor.matmul(o_ps, r32(vh[:, kt, :]), r32(es[:, kt, :]),
                                         start=(kt == 0), stop=(kt == n_kt - 1))
                    se = sepool.tile([1, 512], F32, tag="se", name="seat")
                    for kt in range(n_kt):
                        nc.tensor.matmul(se, r32(ones), r32(es[:, kt, :]),
                                         start=(kt == 0), stop=(kt == n_kt - 1))
                    rec = cpool.tile([1, 512], F32, tag="rec", name="rec")
                    nc.vector.reciprocal(rec, se)
                    rec_bc = cpool.tile([P, 512], F32, tag="recbc", name="recbc")
                    nc.gpsimd.partition_broadcast(rec_bc, rec)
                    och = cpool.tile([P, 512], F32R, tag="och", name="och")
                    nc.vector.tensor_tensor(och, o_ps, rec_bc, ALU.mult)
                    nc.vector.tensor_scalar_add(och, och, bv_sb[:, h:h + 1])
                    nc.sync.dma_start(out=o_d[P * h:P * (h + 1), qsl], in_=och)

        # ------------ P4: partial out^T = wo_slice^T stationary ----------
        with tc.tile_pool(name="p4", bufs=1) as p4pool, \
             tc.tile_pool(name="p4c", bufs=3) as c4pool, \
             tc.tile_pool(name="p4ps", bufs=8, space="PSUM") as ps4pool:
            wo_sb, oc_sb = [], []
            for k in range(4):
                wok = p4pool.tile([P, DIM], F32R, tag=f"wo{k}", name=f"wo{k}")
                nc.sync.dma_start(out=wok, in_=din["woT"][P * k:P * (k + 1), :])
                wo_sb.append(wok)
                ock = p4pool.tile([P, T], F32R, tag=f"oc{k}", name=f"oc{k}")
                nc.sync.dma_start(out=ock, in_=o_d[P * k:P * (k + 1), :])
                oc_sb.append(ock)
            for m in range(DIM // P):
                pss = [ps4pool.tile([P, 512], F32, tag="mm", name=f"pso{nn}")
                       for nn in range(NCH)]
                for nn in range(NCH):
                    for k in range(4):
                        nc.tensor.matmul(
                            pss[nn], r32(wo_sb[k][:, P * m:P * (m + 1)]),
                            r32(oc_sb[k][:, 512 * nn:512 * (nn + 1)]),
                            start=(k == 0), stop=(k == 3))
                for nn in range(NCH):
                    sl = slice(512 * nn, 512 * (nn + 1))
                    ch = c4pool.tile([P, 512], F32, tag="ch", name="cho")
                    nc.vector.tensor_copy(ch, pss[nn])
                    nc.sync.dma_start(out=outT[P * m:P * (m + 1), sl], in_=ch)


# ---------------------------------------------------------------- entry

_NC_CACHE = {}


def _get_nc():
    if "nc" not in _NC_CACHE:
        _NC_CACHE["nc"] = build_bass()
    return _NC_CACHE["nc"]


def _run(inputs, trace=False):
    cores = _host_prep(inputs)
    nc = _get_nc()
    in_maps = [{k: d[k] for k in INPUT_SPECS} for d in cores]
    res = run_bass_kernel_spmd(nc, in_maps, core_ids=list(range(8)), trace=trace)
    outs = [res.results[c]["outT"] for c in range(8)]
    final = np.zeros((B, S, DIM), np.float32)
    wo_b = np.asarray(inputs["wo_b"], np.float32)
    for b in range(B):
        acc = outs[4 * b].copy()
        for g in range(1, HG):
            acc += outs[4 * b + g]
        final[b] = acc.T + wo_b[None, :]
    return final, res


def kernel(**inputs):
    return _run(inputs, trace=False)[0]


def kernel_profiled(**inputs):
    # NTFF profiling hooks are unavailable under this axon client; timing
    # comes from TimelineSim in test.py instead.
    return _run(inputs, trace=False)



# revision 23
# speedup vs baseline: 1.4025x; 1.4025x over previous
"""MLA-style attention (DeepSeek MLA block) on 8 Trainium2 NeuronCores.

Sharding: core c = b*4 + g  (batch b in {0,1}, head-group g in {0..3} = 4 heads).

Key structure (v3):
- The q low-rank path is ABSORBED on the host: Wf = (wq_b * q_norm)[group rows]
  @ wq_a  (768 x 2048). Since the rmsnorm per-token scale r_t commutes through
  the second projection, q = r_t * (Wf x + bf) + b_qb. Each core computes only
  a 384-row slice of wq_a x for the sum-of-squares that defines r_t; the four
  cores of a batch AllGather their 8KB partials (2 collectives, pipelined).
  Collective bounce DMAs must go through gpsimd/SWDGE (HWDGE transfers
  adjacent to a collective get chopped).
- kv path stays two-stage (contraction 512 beats 2048) with kvc replicated.
- k-bias is dropped entirely (softmax is invariant to per-query score shifts);
  v-bias is added after the softmax (rows sum to 1); q biases are applied
  during the on-chip q prep (rope rotation + r_t scaling).
- Inputs x and all first-stage weights are bf16 (same PE throughput as fp32r,
  fp32 accumulation); scores/q/k SBUF tiles stay fp32r. The value path
  (exp(scores), v, attention output, wo) is bf16.
- Softmax denominator via bf16 tree-add + GPSIMD cross-partition reduce
  (keeps the PE free; frees a PSUM bank for score pipelining).
- B-stage for the last chunks is deferred into the phase-II transition to
  keep the PE busy while q-prep loads/rotates.
"""
import ml_dtypes
import numpy as np

import concourse.bass as bass
import concourse.tile as tile
from concourse import bacc, bass_isa, mybir
from concourse.bass_utils import run_bass_kernel_spmd

F32 = mybir.dt.float32
F32R = mybir.dt.float32r
BF16 = mybir.dt.bfloat16
BF_NP = ml_dtypes.bfloat16

B, S, DIM = 2, 2048, 2048
NH = 16
QL, KVL = 1536, 512
NOPE, ROPE, VHD = 128, 64, 128
QK_HD = NOPE + ROPE
EPS = 1e-6
SCALE = QK_HD ** -0.5
HG = 4            # heads per group
T = S
P = 128
KT_D = DIM // P   # 16 contraction tiles over model dim
CH = 256          # phase-I token chunk
NCH = T // CH     # 8 chunks
QSL = QL // HG    # 384 qmid slice rows per core
NKV = 4           # kvc feature tiles (512)
TT = T // P       # 16 token tiles
HT = T // 2
DEFER_B = 5       # chunks >= this get their B-stage after the last sweep


def r32(ap):
    return ap.bitcast(F32R)


# ---------------------------------------------------------------- host side

def _host_prep(inp):
    cos = np.asarray(inp["freqs_cos"], np.float32)   # (S, 32)
    sin = np.asarray(inp["freqs_sin"], np.float32)
    cosT4 = np.ascontiguousarray(np.tile(cos.T, (4, 1)))  # (128, S)
    sinT4 = np.ascontiguousarray(np.tile(sin.T, (4, 1)))

    wqa = np.asarray(inp["wq_a_w"], np.float32)          # (1536, 2048)
    bqa = np.asarray(inp["wq_a_b"], np.float32)
    qn = np.asarray(inp["q_norm_w"], np.float32)
    wqb = np.asarray(inp["wq_b_w"], np.float32) * qn[None, :]   # (3072, 1536)
    bqb = np.asarray(inp["wq_b_b"], np.float32)

    perm_kva = np.concatenate([
        np.arange(KVL),
        KVL + 2 * np.arange(32),
        KVL + 2 * np.arange(32) + 1,
    ])
    wkvaT = np.ascontiguousarray(
        np.asarray(inp["wkv_a_w"], np.float32)[perm_kva].T).astype(BF_NP)
    bkva_p = np.asarray(inp["wkv_a_b"], np.float32)[perm_kva]
    bkva = np.zeros((5, P), np.float32)
    bkva.reshape(-1)[:576] = bkva_p

    kvn = np.asarray(inp["kv_norm_w"], np.float32)
    wkvb = np.asarray(inp["wkv_b_w"], np.float32) * kvn[None, :]
    bkvb = np.asarray(inp["wkv_b_b"], np.float32)
    wo = np.asarray(inp["wo_w"], np.float32)

    # additive causal masks per diagonal sub-tile: -1e30 where (128*di+k) > q
    masks01 = np.zeros((4, P, 512), np.float32)
    for di in range(4):
        kk = np.arange(P)[:, None] + P * di
        qq = np.arange(512)[None, :]
        masks01[di] = np.where(kk > qq, -1e30, 0.0).astype(np.float32)

    cores = []
    for b in range(B):
        xt = np.ascontiguousarray(
            np.asarray(inp["x"], np.float32)[b].T).astype(BF_NP)
        for g in range(HG):
            heads = range(4 * g, 4 * g + 4)
            rows_nope = np.concatenate(
                [np.arange(h * QK_HD, h * QK_HD + NOPE) for h in heads])
            rows_real = np.concatenate(
                [h * QK_HD + NOPE + 2 * np.arange(32) for h in heads])
            rows_imag = np.concatenate(
                [h * QK_HD + NOPE + 2 * np.arange(32) + 1 for h in heads])
            rows_q = np.concatenate([rows_nope, rows_real, rows_imag])

            wf = wqb[rows_q] @ wqa                     # (768, 2048)
            bf = wqb[rows_q] @ bqa                     # (768,)
            bqb_n = bqb[rows_nope]                     # (512,)
            br, bi = bqb[rows_real], bqb[rows_imag]    # (128,) each
            fidx = np.tile(np.arange(32), 4)
            broped_r = (br[:, None] * cos.T[fidx] - bi[:, None] * sin.T[fidx])
            broped_i = (br[:, None] * sin.T[fidx] + bi[:, None] * cos.T[fidx])

            rows_k = np.concatenate(
                [np.arange(h * (NOPE + VHD), h * (NOPE + VHD) + NOPE) for h in heads])
            rows_v = np.concatenate(
                [np.arange(h * (NOPE + VHD) + NOPE, (h + 1) * (NOPE + VHD)) for h in heads])

            # packed per-feature scalars, cols: 0:3 bqa_slice | 3:8 bkva |
            # 8:14 bfused | 14:18 bqb_nope | 18:22 bv | 22 eps
            biases = np.zeros((32, P), np.float32)
            biases[0:3] = bqa[QSL * g: QSL * (g + 1)].reshape(3, P)
            biases[3:8] = bkva
            biases[8:14] = bf.reshape(6, P)
            biases[14:18] = bqb_n.reshape(4, P)
            biases[18:22] = bkvb[rows_v].reshape(4, P)
            biases[22] = EPS
            cores.append(dict(
                xt=xt,
                wqaT_sl=np.ascontiguousarray(
                    wqa[QSL * g: QSL * (g + 1)].T).astype(BF_NP),
                wkvaT=wkvaT,
                wfT=np.ascontiguousarray(wf.T).astype(BF_NP),
                wkvbTk=np.ascontiguousarray(wkvb[rows_k].T).astype(BF_NP),
                wkvbTv=np.ascontiguousarray(wkvb[rows_v].T).astype(BF_NP),
                woT=np.ascontiguousarray(
                    wo[:, 512 * g: 512 * (g + 1)].T).astype(BF_NP),
                biases=biases,
                cosT4=cosT4, sinT4=sinT4,
                broped_r=np.ascontiguousarray(broped_r),
                broped_i=np.ascontiguousarray(broped_i),
                masks01=masks01,
                ones_bf=np.ones((P, 1), BF_NP),
            ))
    return cores


INPUT_SPECS = dict(
    xt=((DIM, T), BF16),
    wqaT_sl=((DIM, QSL), BF16),
    wkvaT=((DIM, 576), BF16),
    wfT=((DIM, 768), BF16),
    wkvbTk=((KVL, 512), BF16),
    wkvbTv=((KVL, 512), BF16),
    woT=((512, DIM), BF16),
    ones_bf=((P, 1), BF16),
    biases=((32, P), F32),
    cosT4=((P, T), F32), sinT4=((P, T), F32),
    broped_r=((P, T), F32), broped_i=((P, T), F32),
    masks01=((4, P, 512), F32),
)


# ---------------------------------------------------------------- device IR

def _blk(w, kt, width, m0=0):
    """AP over DRAM weight w (R, C): (128p rows, kt ktiles, width cols at m0)."""
    rows, cols = w.shape
    return bass.AP(
        tensor=w.tensor, offset=m0,
        ap=[[cols, P], [P * cols, kt], [1, width]],
    )


def _t_view(a2d):
    arows, acols = a2d.shape
    assert acols == P
    return bass.AP(tensor=a2d.tensor, offset=0, ap=[[1, P], [P, arows]])


def build_bass(debug_taps=False):
    nc = bacc.Bacc("TRN2", target_bir_lowering=False, debug=False, num_devices=8)

    din = {name: nc.dram_tensor(name, shape, dt, kind="ExternalInput").ap()
           for name, (shape, dt) in INPUT_SPECS.items()}
    outT = nc.dram_tensor("outT", (DIM, T), F32, kind="ExternalOutput").ap()
    kind = "ExternalOutput" if debug_taps else "Internal"
    scratch = dict(
        qT_d=nc.dram_tensor("qT_d", (768, T), BF16, kind=kind).ap(),
        knope_d=nc.dram_tensor("knope_d", (512, T), F32R, kind=kind).ap(),
        v_d=nc.dram_tensor("v_d", (T, 512), BF16, kind=kind).ap(),
        cc_in=nc.dram_tensor("cc_in", (2, HT), F32).ap(),
        cc_out=nc.dram_tensor("cc_out", (2, 4, HT), F32).ap(),
    )
    if debug_taps:
        scratch["kpe_o"] = nc.dram_tensor("kpe_o", (64, T), F32, kind=kind).ap()
        scratch["rq_o"] = nc.dram_tensor("rq_o", (P, T), F32, kind=kind).ap()

    with tile.TileContext(nc) as tc:
        _emit(tc, din, outT, scratch)

    nc.compile()
    return nc


def _emit(tc, din, outT, scratch):
    nc = tc.nc
    from contextlib import ExitStack
    ALU = mybir.AluOpType
    AF = mybir.ActivationFunctionType
    qT_d, knope_d, v_d = scratch["qT_d"], scratch["knope_d"], scratch["v_d"]
    cc_in, cc_out = scratch["cc_in"], scratch["cc_out"]

    with ExitStack() as outer:
        const = outer.enter_context(tc.tile_pool(name="const", bufs=1))
        ones_bf = const.tile([P, 1], BF16)
        nc.sync.dma_start(out=ones_bf, in_=din["ones_bf"])
        bs = const.tile([P, 32], F32)
        nc.sync.dma_start(out=bs, in_=_t_view(din["biases"]))
        bqa_sb, bkva_sb = bs[:, 0:3], bs[:, 3:8]
        bf_sb, bqbn_sb, bv_sb = bs[:, 8:14], bs[:, 14:18], bs[:, 18:22]
        eps_sb = bs[:, 22:23]
        kpeT = const.tile([64, T], F32R)      # roped shared k_pe [real|imag]
        rkv_tok = const.tile([P, TT], F32)
        rq_bc = const.tile([P, T], F32)       # broadcast 1/rms(q) per token

        # =================== PHASE I: projections ======================
        with ExitStack() as p1:
            wpool = p1.enter_context(tc.tile_pool(name="w1", bufs=1))
            xpool = p1.enter_context(tc.tile_pool(name="p1x", bufs=2))

            # first x chunk ahead of the weight bulk; weights in consumption
            # order, block-granular so each chain starts as its block lands
            x0_sb = xpool.tile([P, KT_D, CH], BF16, tag="x", name="x0")
            nc.sync.dma_start(
                out=x0_sb,
                in_=bass.AP(tensor=din["xt"].tensor, offset=0,
                            ap=[[T, P], [P * T, KT_D], [1, CH]]))
            wqa_sb = []
            for m in range(3):
                t = wpool.tile([P, KT_D, P], BF16, tag=f"wqa{m}", name=f"wqa{m}")
                nc.sync.dma_start(out=t, in_=_blk(din["wqaT_sl"], KT_D, P, P * m))
                wqa_sb.append(t)
            wkva_sb = []
            for m in range(5):
                w = 64 if m == 4 else P
                t = wpool.tile([P, KT_D, w], BF16, tag=f"wkva{m}", name=f"wkva{m}")
                nc.sync.dma_start(out=t, in_=_blk(din["wkvaT"], KT_D, w, P * m))
                wkva_sb.append(t)
            wf_sb = []
            for m in range(6):
                t = wpool.tile([P, KT_D, P], BF16, tag=f"wf{m}", name=f"wf{m}")
                nc.sync.dma_start(out=t, in_=_blk(din["wfT"], KT_D, P, P * m))
                wf_sb.append(t)
            wkk_sb = wpool.tile([P, NKV, 512], BF16)
            nc.sync.dma_start(out=wkk_sb, in_=_blk(din["wkvbTk"], NKV, 512))
            wkv_sb = wpool.tile([P, NKV, 512], BF16)
            nc.sync.dma_start(out=wkv_sb, in_=_blk(din["wkvbTv"], NKV, 512))

            cpool = p1.enter_context(tc.tile_pool(name="p1c", bufs=2))
            kvpool = p1.enter_context(tc.tile_pool(name="p1kv", bufs=1))
            mm = p1.enter_context(tc.tile_pool(name="p1ps", bufs=6, space="PSUM"))
            sspool = p1.enter_context(tc.tile_pool(name="p1ss", bufs=2, space="PSUM"))

            def chunk_sweeps(c):
                csl = slice(CH * c, CH * (c + 1))
                if c == 0:
                    x_sb = x0_sb
                else:
                    x_sb = xpool.tile([P, KT_D, CH], BF16, tag="x", name=f"x{c}")
                    nc.sync.dma_start(
                        out=x_sb,
                        in_=bass.AP(tensor=din["xt"].tensor, offset=CH * c,
                                    ap=[[T, P], [P * T, KT_D], [1, CH]]))

                # ---- sweep 1: qmid-slice (3) + kvc (4) + kpe (1) chains ----
                ps_q = [mm.tile([P, 512], F32, tag="mm", name=f"q{m}")
                        for m in range(3)]
                for m in range(3):
                    for k in range(KT_D):
                        nc.tensor.matmul(
                            ps_q[m][:, :CH],
                            wqa_sb[m][:, k, :],
                            x_sb[:, k, :],
                            start=(k == 0), stop=(k == KT_D - 1))
                ps_kv = [mm.tile([P, 512], F32, tag="mm", name=f"kv{m}")
                         for m in range(4)]
                for m in range(4):
                    for k in range(KT_D):
                        nc.tensor.matmul(
                            ps_kv[m][:, :CH],
                            wkva_sb[m][:, k, :],
                            x_sb[:, k, :],
                            start=(k == 0), stop=(k == KT_D - 1))
                ps_kpe = mm.tile([P, 512], F32, tag="mm", name="kpe")
                for k in range(KT_D):
                    nc.tensor.matmul(
                        ps_kpe[:64, :CH],
                        wkva_sb[4][:, k, :],
                        x_sb[:, k, :],
                        start=(k == 0), stop=(k == KT_D - 1))

                # ---- sweep-1 evictions ----
                sq = []   # squared qmid slices (bf16: feeds a bf16 sum-MM)
                for m in range(3):
                    t = cpool.tile([P, CH], BF16, tag=f"sq{m}", name=f"sq{m}", bufs=1)
                    nc.scalar.activation(t, ps_q[m][:, :CH], AF.Square,
                                         bias=bqa_sb[:, m:m + 1])
                    sq.append(t)
                kvc_sb, kvsq = [], []
                for m in range(4):
                    t = kvpool.tile([P, CH], BF16, tag=f"kvc{m}", name=f"kvc{m}",
                                    bufs=4)
                    nc.vector.tensor_scalar(t, ps_kv[m][:, :CH],
                                            bkva_sb[:, m:m + 1], None, ALU.add)
                    kvc_sb.append(t)
                    t2 = cpool.tile([P, CH], BF16, tag=f"kvsq{m}", name=f"kvsq{m}",
                                    bufs=1)
                    nc.scalar.activation(t2, ps_kv[m][:, :CH], AF.Square,
                                         bias=bkva_sb[:, m:m + 1])
                    kvsq.append(t2)
                kpe_raw = cpool.tile([64, CH], F32, tag="kpr", name="kpr", bufs=1)
                nc.scalar.activation(kpe_raw, ps_kpe[:64, :CH], AF.Identity,
                                     bias=bkva_sb[0:64, 4:5])
                # rope-rotate k_pe into kpeT[:, csl]; imag half of the input
                # is DMA-shifted down to partitions 0..31 first
                c32 = cpool.tile([32, CH], F32, tag="c32", name="c32", bufs=1)
                nc.sync.dma_start(out=c32, in_=din["cosT4"][0:32, csl])
                s32 = cpool.tile([32, CH], F32, tag="s32", name="s32", bufs=1)
                nc.sync.dma_start(out=s32, in_=din["sinT4"][0:32, csl])
                xr = kpe_raw[0:32, :]
                xi = cpool.tile([32, CH], F32, tag="xikp", name="xikp", bufs=1)
                nc.sync.dma_start(out=xi, in_=kpe_raw[32:64, :])
                t2_ = cpool.tile([32, CH], F32, tag="t2", name="t2", bufs=1)
                yikp = cpool.tile([32, CH], F32R, tag="yikp", name="yikp", bufs=1)
                nc.vector.tensor_tensor(kpeT[0:32, csl], xr, c32, ALU.mult)
                nc.vector.tensor_tensor(t2_, xi, s32, ALU.mult)
                nc.vector.tensor_tensor(kpeT[0:32, csl], kpeT[0:32, csl], t2_,
                                        ALU.subtract)
                nc.vector.tensor_tensor(yikp, xr, s32, ALU.mult)
                nc.vector.tensor_tensor(t2_, xi, c32, ALU.mult)
                nc.vector.tensor_tensor(yikp, yikp, t2_, ALU.add)
                nc.sync.dma_start(out=kpeT[32:64, csl], in_=yikp)

                # ---- sweep 2: fused-q (6 chains) + sumsq chains ----
                ps_fq = [mm.tile([P, 512], F32, tag="mm", name=f"fq{m}")
                         for m in range(6)]
                for m in range(6):
                    for k in range(KT_D):
                        nc.tensor.matmul(
                            ps_fq[m][:, :CH],
                            wf_sb[m][:, k, :],
                            x_sb[:, k, :],
                            start=(k == 0), stop=(k == KT_D - 1))
                ps_qss = sspool.tile([1, 512], F32, tag="ss", name="qss")
                for m in range(3):
                    nc.tensor.matmul(ps_qss[:, :CH], ones_bf, sq[m],
                                     start=(m == 0), stop=(m == 2))
                ps_kss = sspool.tile([1, 512], F32, tag="ss", name="kss")
                for m in range(4):
                    nc.tensor.matmul(ps_kss[:, :CH], ones_bf, kvsq[m],
                                     start=(m == 0), stop=(m == 3))

                # ---- sweep-2 evictions ----
                qch = cpool.tile([P, 6, CH], BF16, tag="qch", name="qch", bufs=1)
                for m in range(6):
                    nc.scalar.activation(qch[:, m, :], ps_fq[m][:, :CH],
                                         AF.Identity, bias=bf_sb[:, m:m + 1])
                nc.sync.dma_start(
                    out=bass.AP(tensor=qT_d.tensor, offset=CH * c,
                                ap=[[T, P], [P * T, 6], [1, CH]]),
                    in_=qch)
                # q-sumsq partial straight to the collective input buffer
                qssv = cpool.tile([1, CH], F32, tag="qssv", name="qssv", bufs=1)
                nc.vector.tensor_copy(qssv, ps_qss[:, :CH])
                nc.gpsimd.dma_start(
                    out=bass.AP(tensor=cc_in.tensor, offset=CH * c,
                                ap=[[CH, 1], [1, CH]]),
                    in_=qssv)
                # rkv for this chunk
                srt = cpool.tile([1, CH], F32, tag="srt", name="srt", bufs=1)
                nc.scalar.activation(srt, ps_kss[:, :CH], AF.Sqrt,
                                     bias=eps_sb[0:1, :], scale=1.0 / KVL)
                rkv_c = cpool.tile([1, CH], F32, tag="rkvc", name="rkvc", bufs=1)
                nc.vector.reciprocal(rkv_c, srt)
                rkv_bc = cpool.tile([P, CH], F32, tag="rkvbc", name="rkvbc",
                                    bufs=4)
                nc.gpsimd.partition_broadcast(rkv_bc, rkv_c)
                for tt in range(2):
                    nc.sync.dma_start(
                        out=rkv_tok[:, 2 * c + tt: 2 * c + tt + 1],
                        in_=rkv_c[:, P * tt:P * (tt + 1)])
                return kvc_sb, rkv_bc

            def chunk_b(c, kvc_sb, rkv_bc):
                ps_kn = [mm.tile([P, 512], F32, tag="mm", name=f"kn{m}")
                         for m in range(4)]
                for m in range(4):
                    for k in range(NKV):
                        nc.tensor.matmul(
                            ps_kn[m][:, :CH],
                            wkk_sb[:, k, P * m:P * (m + 1)],
                            kvc_sb[k],
                            start=(k == 0), stop=(k == NKV - 1))
                ps_v = [mm.tile([P, 512], F32, tag="mm", name=f"v{tt}")
                        for tt in range(2)]
                for tt in range(2):
                    for k in range(NKV):
                        nc.tensor.matmul(
                            ps_v[tt],
                            kvc_sb[k][:, P * tt:P * (tt + 1)],
                            wkv_sb[:, k, :],
                            start=(k == 0), stop=(k == NKV - 1))
                kn_ch = cpool.tile([P, 4, CH], F32R, tag="knch", name="knch",
                                   bufs=2)
                for m in range(4):
                    nc.vector.tensor_tensor(kn_ch[:, m, :], ps_kn[m][:, :CH],
                                            rkv_bc, ALU.mult)
                nc.sync.dma_start(
                    out=bass.AP(tensor=knope_d.tensor, offset=CH * c,
                                ap=[[T, P], [P * T, 4], [1, CH]]),
                    in_=kn_ch)
                v_ch = cpool.tile([P, 2, 512], BF16, tag="vch", name="vch",
                                  bufs=2)
                for tt in range(2):
                    nc.scalar.activation(
                        v_ch[:, tt, :], ps_v[tt], AF.Copy,
                        scale=rkv_tok[:, 2 * c + tt: 2 * c + tt + 1])
                nc.sync.dma_start(
                    out=bass.AP(tensor=v_d.tensor, offset=512 * CH * c,
                                ap=[[512, P], [512 * P, 2], [1, 512]]),
                    in_=v_ch)

            def emit_ag_start(half):
                cc_out_ap = bass.AP(tensor=cc_out.tensor, offset=half * 4 * HT,
                                    ap=[[HT, 4], [1, HT]])
                nc.gpsimd.collective_compute(
                    "AllGather",
                    ALU.bypass,
                    replica_groups=[[0, 1, 2, 3], [4, 5, 6, 7]],
                    ins=[cc_in[half:half + 1, :]],
                    outs=[cc_out_ap],
                )

            def emit_ag_finish(half, use_sync_dma):
                for qc in range(4):
                    off = half * 4 * HT + CH * qc
                    gath = cpool.tile([4, CH], F32, tag="gath", name="gath",
                                      bufs=1)
                    dma = nc.sync.dma_start if use_sync_dma else nc.gpsimd.dma_start
                    dma(out=gath,
                        in_=bass.AP(tensor=cc_out.tensor, offset=off,
                                    ap=[[HT, 4], [1, CH]]))
                    asum = cpool.tile([4, CH], F32, tag="asum", name="asum",
                                      bufs=1)
                    nc.gpsimd.partition_all_reduce(
                        asum, gath, channels=4, reduce_op=bass_isa.ReduceOp.add)
                    srt = cpool.tile([1, CH], F32, tag="qsrt", name="qsrt",
                                     bufs=1)
                    nc.scalar.activation(srt, asum[0:1, :], AF.Sqrt,
                                         bias=eps_sb[0:1, :], scale=1.0 / QL)
                    rqh = cpool.tile([1, CH], F32, tag="rqh", name="rqh",
                                     bufs=1)
                    nc.vector.reciprocal(rqh, srt)
                    gsl = slice(HT * half + CH * qc, HT * half + CH * (qc + 1))
                    nc.gpsimd.partition_broadcast(rq_bc[:, gsl], rqh)

            deferred = {}
            for c in range(NCH):
                kvc_sb, rkv_bc = chunk_sweeps(c)
                if c < DEFER_B:
                    chunk_b(c, kvc_sb, rkv_bc)
                else:
                    deferred[c] = (kvc_sb, rkv_bc)
                if c == 3:
                    emit_ag_start(0)
                if c == 5:
                    emit_ag_finish(0, use_sync_dma=True)
            emit_ag_start(1)
            # deferred B-work lands in the PE queue right where phase II
            # would otherwise starve waiting for q-prep
            for c in sorted(deferred):
                chunk_b(c, *deferred[c])

            # the gpsimd-side finish for the second half runs while the PE
            # chews on the deferred B-chunks
            emit_ag_finish(1, use_sync_dma=False)

        # =================== PHASE II: q prep + attention + out ==========
        with ExitStack() as p2:
            apool = p2.enter_context(tc.tile_pool(name="p2a", bufs=1))
            qpool = p2.enter_context(tc.tile_pool(name="p2q", bufs=1))
            prep = p2.enter_context(tc.tile_pool(name="p2p", bufs=1))
            hpool = p2.enter_context(tc.tile_pool(name="p2h", bufs=2))
            cpool = p2.enter_context(tc.tile_pool(name="p2c", bufs=2))
            espool = p2.enter_context(tc.tile_pool(name="p2e", bufs=2))
            opool = p2.enter_context(tc.tile_pool(name="p2osb", bufs=16))
            oute = p2.enter_context(tc.tile_pool(name="p2oute", bufs=3))
            mm2 = p2.enter_context(tc.tile_pool(name="p2s", bufs=4, space="PSUM"))
            omm = p2.enter_context(tc.tile_pool(name="p2o", bufs=2, space="PSUM"))
            p4mm = p2.enter_context(tc.tile_pool(name="p2p4", bufs=2, space="PSUM"))

            def load_head(h):
                kn = hpool.tile([P, T], F32R, tag="kn", name=f"kn{h}")
                nc.sync.dma_start(out=kn, in_=knope_d[P * h:P * (h + 1), :])
                vh = hpool.tile([P, TT, P], BF16, tag="vh", name=f"vh{h}")
                nc.sync.dma_start(
                    out=vh,
                    in_=bass.AP(tensor=v_d.tensor, offset=P * h,
                                ap=[[512, P], [P * 512, TT], [1, P]]))
                return kn, vh

            def prep_alloc(pair):
                qn_f, qr_f = {}, {}
                for i in range(2):
                    h = 2 * pair + i
                    qn_f[h] = qpool.tile([P, T], F32R, tag=f"qn{i}",
                                         name=f"qnf{h}")
                    qr_f[h] = qpool.tile([64, T], F32R, tag=f"qr{i}",
                                         name=f"qrf{h}")
                return qn_f, qr_f

            def prep_pair_half(pair, half, qn_f, qr_f, cosf, sinf):
                hsl = slice(HT * half, HT * (half + 1))
                for i in range(2):
                    h = 2 * pair + i
                    tmp = prep.tile([P, HT], BF16, tag="qtmp", name="qtmp")
                    nc.sync.dma_start(
                        out=tmp, in_=qT_d[P * h:P * (h + 1), hsl])
                    nc.vector.tensor_tensor(qn_f[h][:, hsl], tmp,
                                            rq_bc[:, hsl], ALU.mult)
                    nc.vector.tensor_scalar(
                        qn_f[h][:, hsl], qn_f[h][:, hsl],
                        bqbn_sb[:, h:h + 1], None, ALU.add)
                xr = prep.tile([64, HT], BF16, tag="xr", name="xr")
                nc.sync.dma_start(
                    out=xr, in_=qT_d[512 + 64 * pair:512 + 64 * pair + 64, hsl])
                xi = prep.tile([64, HT], BF16, tag="xi", name="xi")
                nc.sync.dma_start(
                    out=xi, in_=qT_d[640 + 64 * pair:640 + 64 * pair + 64, hsl])
                bro_r = prep.tile([64, HT], F32, tag="bror", name="bror")
                nc.sync.dma_start(
                    out=bro_r,
                    in_=din["broped_r"][64 * pair:64 * pair + 64, hsl])
                bro_i = prep.tile([64, HT], F32, tag="broi", name="broi")
                nc.sync.dma_start(
                    out=bro_i,
                    in_=din["broped_i"][64 * pair:64 * pair + 64, hsl])
                c_, s_ = cosf[:, hsl], sinf[:, hsl]
                t1 = prep.tile([64, HT], F32, tag="t1", name="t1")
                t2 = prep.tile([64, HT], F32, tag="t2", name="t2")
                yr = prep.tile([64, HT], F32R, tag="yr", name="yr")
                yi = prep.tile([64, HT], F32R, tag="yi", name="yi")
                nc.vector.tensor_tensor(t1, xr, c_, ALU.mult)
                nc.vector.tensor_tensor(t2, xi, s_, ALU.mult)
                nc.vector.tensor_tensor(yr, t1, t2, ALU.subtract)
                nc.vector.tensor_tensor(t1, xr, s_, ALU.mult)
                nc.vector.tensor_tensor(t2, xi, c_, ALU.mult)
                nc.vector.tensor_tensor(yi, t1, t2, ALU.add)
                nc.vector.tensor_tensor(yr, yr, rq_bc[0:64, hsl], ALU.mult)
                nc.vector.tensor_tensor(yr, yr, bro_r, ALU.add)
                nc.vector.tensor_tensor(yi, yi, rq_bc[0:64, hsl], ALU.mult)
                nc.vector.tensor_tensor(yi, yi, bro_i, ALU.add)
                for i in range(2):
                    h = 2 * pair + i
                    nc.sync.dma_start(out=qr_f[h][0:32, hsl],
                                      in_=yr[32 * i:32 * i + 32, :])
                    nc.sync.dma_start(out=qr_f[h][32:64, hsl],
                                      in_=yi[32 * i:32 * i + 32, :])

            o_sb = {}

            def emit_p4(qch):
                qsl = slice(512 * qch, 512 * (qch + 1))
                for m in range(DIM // P):
                    ps = p4mm.tile([P, 512], F32, tag="p4", name="p4")
                    for hh in range(4):
                        nc.tensor.matmul(
                            ps, wo_sb[:, hh, P * m:P * (m + 1)],
                            o_sb[(hh, qch)],
                            start=(hh == 0), stop=(hh == 3))
                    ot = oute.tile([P, 512], F32, tag="ot", name="ot")
                    nc.scalar.activation(ot, ps, AF.Copy)
                    nc.sync.dma_start(out=outT[P * m:P * (m + 1), qsl], in_=ot)

            def attention_qch(h, kn, vh, qn_f, qr_f, qch, with_p4):
                qsl = slice(512 * qch, 512 * (qch + 1))
                n_kt = 4 * (qch + 1)
                es = espool.tile([P, TT, 512], BF16, tag="es", name="es")
                for kt in range(n_kt):
                    ps = mm2.tile([P, 512], F32, tag="s", name="s")
                    nc.tensor.matmul(ps, r32(kn[:, P * kt:P * (kt + 1)]),
                                     r32(qn_f[h][:, qsl]),
                                     start=True, stop=False)
                    nc.tensor.matmul(ps, r32(kpeT[:, P * kt:P * (kt + 1)]),
                                     r32(qr_f[h][:, qsl]),
                                     start=False, stop=True)
                    di = kt - 4 * qch
                    if di >= 0:
                        nc.vector.tensor_tensor(ps, ps, masks_sb[:, di, :],
                                                ALU.add)
                    nc.scalar.activation(es[:, kt, :], ps, AF.Exp,
                                         scale=SCALE)
                o_ps = omm.tile([P, 512], F32, tag="o", name="o")
                for kt in range(n_kt):
                    nc.tensor.matmul(o_ps, vh[:, kt, :], es[:, kt, :],
                                     start=(kt == 0), stop=(kt == n_kt - 1))
                # softmax denominator off the PE: bf16 tree-sum over the kt
                # tiles, then a cross-partition reduce on GPSIMD
                ses = cpool.tile([P, 512], BF16, tag="ses", name="ses")
                nc.vector.tensor_tensor(ses, es[:, 0, :], es[:, 1, :], ALU.add)
                for kt in range(2, n_kt):
                    nc.vector.tensor_tensor(ses, ses, es[:, kt, :], ALU.add)
                sesum = cpool.tile([P, 512], F32, tag="sesum", name="sesum")
                nc.gpsimd.partition_all_reduce(sesum, ses, channels=P,
                                               reduce_op=bass_isa.ReduceOp.add)
                rec = cpool.tile([P, 512], F32, tag="rec", name="rec")
                nc.vector.reciprocal(rec, sesum)
                och = opool.tile([P, 512], BF16, tag="och", name=f"o{h}{qch}")
                nc.vector.tensor_tensor(och, o_ps, rec, ALU.mult)
                nc.vector.tensor_scalar(och, och, bv_sb[:, h:h + 1], None,
                                        ALU.add)
                o_sb[(h, qch)] = och
                if with_p4:
                    emit_p4(qch)

            # transition-critical loads first: q-prep inputs for pair 0,
            # then h0's k/v, then the rest
            cosf = apool.tile([64, T], F32)
            nc.sync.dma_start(out=cosf, in_=din["cosT4"][0:64, :])
            sinf = apool.tile([64, T], F32)
            nc.sync.dma_start(out=sinf, in_=din["sinT4"][0:64, :])
            qn_f, qr_f = prep_alloc(0)
            prep_pair_half(0, 0, qn_f, qr_f, cosf, sinf)
            kn0, vh0 = load_head(0)
            masks_sb = apool.tile([P, 4, 512], F32)
            nc.sync.dma_start(
                out=masks_sb,
                in_=bass.AP(tensor=din["masks01"].tensor, offset=0,
                            ap=[[512, P], [P * 512, 4], [1, 512]]))
            attention_qch(0, kn0, vh0, qn_f, qr_f, 0, False)
            attention_qch(0, kn0, vh0, qn_f, qr_f, 1, False)
            prep_pair_half(0, 1, qn_f, qr_f, cosf, sinf)
            kn1, vh1 = load_head(1)
            attention_qch(0, kn0, vh0, qn_f, qr_f, 2, False)
            attention_qch(0, kn0, vh0, qn_f, qr_f, 3, False)
            for qch in range(4):
                attention_qch(1, kn1, vh1, qn_f, qr_f, qch, False)
            wo_sb = apool.tile([P, 4, T], BF16)
            nc.sync.dma_start(out=wo_sb, in_=_blk(din["woT"], 4, T))
            qn_f2, qr_f2 = prep_alloc(1)
            prep_pair_half(1, 0, qn_f2, qr_f2, cosf, sinf)
            prep_pair_half(1, 1, qn_f2, qr_f2, cosf, sinf)
            kn2, vh2 = load_head(2)
            for qch in range(4):
                attention_qch(2, kn2, vh2, qn_f2, qr_f2, qch, False)
            kn3, vh3 = load_head(3)
            for qch in range(4):
                attention_qch(3, kn3, vh3, qn_f2, qr_f2, qch, True)

            if "kpe_o" in scratch:
                nc.sync.dma_start(out=scratch["kpe_o"], in_=kpeT.bitcast(F32))
                nc.sync.dma_start(out=scratch["rq_o"], in_=rq_bc)


# ---------------------------------------------------------------- entry

_NC_CACHE = {}


def _get_nc():
    if "nc" not in _NC_CACHE:
        _NC_CACHE["nc"] = build_bass()
    return _NC_CACHE["nc"]


def _run(inputs, trace=False):
    cores = _host_prep(inputs)
    nc = _get_nc()
    in_maps = [{k: d[k] for k in INPUT_SPECS} for d in cores]
    res = run_bass_kernel_spmd(nc, in_maps, core_ids=list(range(8)), trace=trace)
    outs = [res.results[c]["outT"] for c in range(8)]
    final = np.zeros((B, S, DIM), np.float32)
    wo_b = np.asarray(inputs["wo_b"], np.float32)
    for b in range(B):
        acc = outs[4 * b].copy()
        for g in range(1, HG):
            acc += outs[4 * b + g]
        final[b] = acc.T + wo_b[None, :]
    return final, res


def kernel(**inputs):
    return _run(inputs, trace=False)[0]


def kernel_profiled(**inputs):
    return _run(inputs, trace=False)


# revision 42
# speedup vs baseline: 1.5070x; 1.0745x over previous
"""MLA-style attention (DeepSeek MLA block) on 8 Trainium2 NeuronCores.

Sharding: core c = b*4 + g  (batch b in {0,1}, head-group g in {0..3} = 4 heads).

Key structure (v3):
- The q low-rank path is ABSORBED on the host: Wf = (wq_b * q_norm)[group rows]
  @ wq_a  (768 x 2048). Since the rmsnorm per-token scale r_t commutes through
  the second projection, q = r_t * (Wf x + bf) + b_qb. Each core computes only
  a 384-row slice of wq_a x for the sum-of-squares that defines r_t; the four
  cores of a batch AllGather their 8KB partials (2 collectives, pipelined).
  Collective bounce DMAs must go through gpsimd/SWDGE (HWDGE transfers
  adjacent to a collective get chopped).
- kv path stays two-stage (contraction 512 beats 2048) with kvc replicated.
- k-bias is dropped entirely (softmax is invariant to per-query score shifts);
  v-bias is added after the softmax (rows sum to 1); q biases are applied
  during the on-chip q prep (rope rotation + r_t scaling).
- Inputs x and all first-stage weights are bf16 (same PE throughput as fp32r,
  fp32 accumulation); scores/q/k SBUF tiles stay fp32r. The value path
  (exp(scores), v, attention output, wo) is bf16.
- Softmax denominator via bf16 tree-add + GPSIMD cross-partition reduce
  (keeps the PE free; frees a PSUM bank for score pipelining).
- B-stage for the last chunks is deferred into the phase-II transition to
  keep the PE busy while q-prep loads/rotates.
"""
import ml_dtypes
import numpy as np

import concourse.bass as bass
import concourse.tile as tile
from concourse import bacc, bass_isa, mybir
from concourse.bass_utils import run_bass_kernel_spmd

F32 = mybir.dt.float32
F32R = mybir.dt.float32r
BF16 = mybir.dt.bfloat16
BF_NP = ml_dtypes.bfloat16

B, S, DIM = 2, 2048, 2048
NH = 16
QL, KVL = 1536, 512
NOPE, ROPE, VHD = 128, 64, 128
QK_HD = NOPE + ROPE
EPS = 1e-6
SCALE = QK_HD ** -0.5
HG = 4            # heads per group
T = S
P = 128
KT_D = DIM // P   # 16 contraction tiles over model dim
CH = 256          # phase-I token chunk
NCH = T // CH     # 8 chunks
QSL = QL // HG    # 384 qmid slice rows per core
NKV = 4           # kvc feature tiles (512)
TT = T // P       # 16 token tiles
HT = T // 2
DEFER_B = 5       # chunks >= this get their B-stage after the last sweep


def r32(ap):
    return ap.bitcast(F32R)


# ---------------------------------------------------------------- host side

def _host_prep(inp):
    cos = np.asarray(inp["freqs_cos"], np.float32)   # (S, 32)
    sin = np.asarray(inp["freqs_sin"], np.float32)
    cosT4 = np.ascontiguousarray(np.tile(cos.T, (4, 1))).astype(BF_NP)
    sinT4 = np.ascontiguousarray(np.tile(sin.T, (4, 1))).astype(BF_NP)

    wqa = np.asarray(inp["wq_a_w"], np.float32)          # (1536, 2048)
    bqa = np.asarray(inp["wq_a_b"], np.float32)
    qn = np.asarray(inp["q_norm_w"], np.float32)
    wqb = np.asarray(inp["wq_b_w"], np.float32) * qn[None, :]   # (3072, 1536)
    bqb = np.asarray(inp["wq_b_b"], np.float32)

    perm_kva = np.concatenate([
        np.arange(KVL),
        KVL + 2 * np.arange(32),
        KVL + 2 * np.arange(32) + 1,
    ])
    wkvaT = np.ascontiguousarray(
        np.asarray(inp["wkv_a_w"], np.float32)[perm_kva].T).astype(BF_NP)
    bkva_p = np.asarray(inp["wkv_a_b"], np.float32)[perm_kva]
    bkva = np.zeros((5, P), np.float32)
    bkva.reshape(-1)[:576] = bkva_p

    kvn = np.asarray(inp["kv_norm_w"], np.float32)
    wkvb = np.asarray(inp["wkv_b_w"], np.float32) * kvn[None, :]
    bkvb = np.asarray(inp["wkv_b_b"], np.float32)
    wo = np.asarray(inp["wo_w"], np.float32)

    # additive causal masks per diagonal sub-tile: -1e30 where (128*di+k) > q
    masks01 = np.zeros((4, P, 512), np.float32)
    for di in range(4):
        kk = np.arange(P)[:, None] + P * di
        qq = np.arange(512)[None, :]
        masks01[di] = np.where(kk > qq, -1e30, 0.0).astype(np.float32)

    cores = []
    for b in range(B):
        xt = np.ascontiguousarray(
            np.asarray(inp["x"], np.float32)[b].T).astype(BF_NP)
        for g in range(HG):
            heads = range(4 * g, 4 * g + 4)
            rows_nope = np.concatenate(
                [np.arange(h * QK_HD, h * QK_HD + NOPE) for h in heads])
            rows_real = np.concatenate(
                [h * QK_HD + NOPE + 2 * np.arange(32) for h in heads])
            rows_imag = np.concatenate(
                [h * QK_HD + NOPE + 2 * np.arange(32) + 1 for h in heads])
            rows_q = np.concatenate([rows_nope, rows_real, rows_imag])

            wf = wqb[rows_q] @ wqa                     # (768, 2048)
            bf = wqb[rows_q] @ bqa                     # (768,)
            bqb_n = bqb[rows_nope]                     # (512,)
            br, bi = bqb[rows_real], bqb[rows_imag]    # (128,) each
            fidx = np.tile(np.arange(32), 4)
            broped_r = (br[:, None] * cos.T[fidx] - bi[:, None] * sin.T[fidx])
            broped_i = (br[:, None] * sin.T[fidx] + bi[:, None] * cos.T[fidx])

            rows_k = np.concatenate(
                [np.arange(h * (NOPE + VHD), h * (NOPE + VHD) + NOPE) for h in heads])
            rows_v = np.concatenate(
                [np.arange(h * (NOPE + VHD) + NOPE, (h + 1) * (NOPE + VHD)) for h in heads])

            # packed per-feature scalars, cols: 0:3 bqa_slice | 3:8 bkva |
            # 8:14 bfused | 14:18 bqb_nope | 18:22 bv | 22 eps
            biases = np.zeros((32, P), np.float32)
            biases[0:3] = bqa[QSL * g: QSL * (g + 1)].reshape(3, P)
            biases[3:8] = bkva
            biases[8:14] = bf.reshape(6, P)
            biases[14:18] = bqb_n.reshape(4, P)
            biases[18:22] = bkvb[rows_v].reshape(4, P)
            biases[22] = EPS
            cores.append(dict(
                xt=xt,
                wqaT_sl=np.ascontiguousarray(
                    wqa[QSL * g: QSL * (g + 1)].T).astype(BF_NP),
                wkvaT=wkvaT,
                wfT=np.ascontiguousarray(wf.T).astype(BF_NP),
                wkvbTk=np.ascontiguousarray(wkvb[rows_k].T).astype(BF_NP),
                wkvbTv=np.ascontiguousarray(wkvb[rows_v].T).astype(BF_NP),
                woT=np.ascontiguousarray(
                    wo[:, 512 * g: 512 * (g + 1)].T).astype(BF_NP),
                biases=biases,
                cosT4=cosT4, sinT4=sinT4,
                broped_r=np.ascontiguousarray(broped_r).astype(BF_NP),
                broped_i=np.ascontiguousarray(broped_i).astype(BF_NP),
                masks01=masks01,
                ones_bf=np.ones((P, 1), BF_NP),
            ))
    return cores


INPUT_SPECS = dict(
    xt=((DIM, T), BF16),
    wqaT_sl=((DIM, QSL), BF16),
    wkvaT=((DIM, 576), BF16),
    wfT=((DIM, 768), BF16),
    wkvbTk=((KVL, 512), BF16),
    wkvbTv=((KVL, 512), BF16),
    woT=((512, DIM), BF16),
    ones_bf=((P, 1), BF16),
    biases=((32, P), F32),
    cosT4=((P, T), BF16), sinT4=((P, T), BF16),
    broped_r=((P, T), BF16), broped_i=((P, T), BF16),
    masks01=((4, P, 512), F32),
)


# ---------------------------------------------------------------- device IR

def _blk(w, kt, width, m0=0):
    """AP over DRAM weight w (R, C): (128p rows, kt ktiles, width cols at m0)."""
    rows, cols = w.shape
    return bass.AP(
        tensor=w.tensor, offset=m0,
        ap=[[cols, P], [P * cols, kt], [1, width]],
    )


def _t_view(a2d):
    arows, acols = a2d.shape
    assert acols == P
    return bass.AP(tensor=a2d.tensor, offset=0, ap=[[1, P], [P, arows]])


def build_bass(debug_taps=False):
    nc = bacc.Bacc("TRN2", target_bir_lowering=False, debug=False, num_devices=8)

    din = {name: nc.dram_tensor(name, shape, dt, kind="ExternalInput").ap()
           for name, (shape, dt) in INPUT_SPECS.items()}
    outT = nc.dram_tensor("outT", (DIM, T), F32, kind="ExternalOutput").ap()
    kind = "ExternalOutput" if debug_taps else "Internal"
    scratch = dict(
        qT_d=nc.dram_tensor("qT_d", (768, T), BF16, kind=kind).ap(),
        knope_d=nc.dram_tensor("knope_d", (512, T), F32R, kind=kind).ap(),
        v_d=nc.dram_tensor("v_d", (T, 512), BF16, kind=kind).ap(),
        cc_in=nc.dram_tensor("cc_in", (2, HT), F32).ap(),
        cc_out=nc.dram_tensor("cc_out", (2, 4, HT), F32).ap(),
    )
    if debug_taps:
        scratch["kpe_o"] = nc.dram_tensor("kpe_o", (64, T), F32, kind=kind).ap()
        scratch["rq_o"] = nc.dram_tensor("rq_o", (P, T), F32, kind=kind).ap()

    with tile.TileContext(nc) as tc:
        _emit(tc, din, outT, scratch)

    nc.compile()
    return nc


def _emit(tc, din, outT, scratch):
    nc = tc.nc
    from contextlib import ExitStack
    ALU = mybir.AluOpType
    AF = mybir.ActivationFunctionType
    qT_d, knope_d, v_d = scratch["qT_d"], scratch["knope_d"], scratch["v_d"]
    cc_in, cc_out = scratch["cc_in"], scratch["cc_out"]

    with ExitStack() as outer:
        const = outer.enter_context(tc.tile_pool(name="const", bufs=1))
        ones_bf = const.tile([P, 1], BF16)
        nc.sync.dma_start(out=ones_bf, in_=din["ones_bf"])
        bs = const.tile([P, 32], F32)
        nc.sync.dma_start(out=bs, in_=_t_view(din["biases"]))
        bqa_sb, bkva_sb = bs[:, 0:3], bs[:, 3:8]
        bf_sb, bqbn_sb, bv_sb = bs[:, 8:14], bs[:, 14:18], bs[:, 18:22]
        eps_sb = bs[:, 22:23]
        kpeT = const.tile([64, T], BF16)      # roped shared k_pe [real|imag]
        rkv_tok = const.tile([P, TT], F32)
        rq_bc = const.tile([P, T], F32)       # broadcast 1/rms(q) per token
        apool = outer.enter_context(tc.tile_pool(name="p2a", bufs=1))
        qpool = outer.enter_context(tc.tile_pool(name="p2q", bufs=1))
        prep = outer.enter_context(tc.tile_pool(name="p2p", bufs=1))
        agp = outer.enter_context(tc.tile_pool(name="agp", bufs=1))
        cc_out_t = scratch["cc_out"]

        def emit_ag_finish(half, use_sync_dma):
            AF = mybir.ActivationFunctionType
            for qc in range(4):
                off = half * 4 * HT + CH * qc
                gath = agp.tile([4, CH], F32, tag="gath", name="gath", bufs=1)
                dma = nc.sync.dma_start if use_sync_dma else nc.gpsimd.dma_start
                dma(out=gath,
                    in_=bass.AP(tensor=cc_out_t.tensor, offset=off,
                                ap=[[HT, 4], [1, CH]]))
                asum = agp.tile([4, CH], F32, tag="asum", name="asum", bufs=1)
                nc.gpsimd.partition_all_reduce(
                    asum, gath, channels=4, reduce_op=bass_isa.ReduceOp.add)
                srt = agp.tile([1, CH], F32, tag="qsrt", name="qsrt", bufs=1)
                nc.scalar.activation(srt, asum[0:1, :], AF.Sqrt,
                                     bias=eps_sb[0:1, :], scale=1.0 / QL)
                rqh = agp.tile([1, CH], F32, tag="rqh", name="rqh", bufs=1)
                nc.vector.reciprocal(rqh, srt)
                gsl = slice(HT * half + CH * qc, HT * half + CH * (qc + 1))
                nc.gpsimd.partition_broadcast(rq_bc[:, gsl], rqh)

        def prep_alloc(pool, pair):
            qn_f, qr_f = {}, {}
            for i in range(2):
                h = 2 * pair + i
                qn_f[h] = pool.tile([P, T], F32R, tag=f"qn{i}", name=f"qnf{h}")
                qr_f[h] = pool.tile([64, T], BF16, tag=f"qr{i}", name=f"qrf{h}")
            return qn_f, qr_f

        def prep_pair_half(pair, half, qn_f, qr_f, cosf, sinf):
            ALU = mybir.AluOpType
            qT_d = scratch["qT_d"]
            hsl = slice(HT * half, HT * (half + 1))
            for i in range(2):
                h = 2 * pair + i
                tmp = prep.tile([P, HT], BF16, tag="qtmp", name="qtmp")
                nc.sync.dma_start(out=tmp, in_=qT_d[P * h:P * (h + 1), hsl])
                nc.vector.tensor_tensor(qn_f[h][:, hsl], tmp,
                                        rq_bc[:, hsl], ALU.mult)
                nc.vector.tensor_scalar(
                    qn_f[h][:, hsl], qn_f[h][:, hsl],
                    bqbn_sb[:, h:h + 1], None, ALU.add)
            xr = prep.tile([64, HT], BF16, tag="xr", name="xr")
            nc.sync.dma_start(
                out=xr, in_=qT_d[512 + 64 * pair:512 + 64 * pair + 64, hsl])
            xi = prep.tile([64, HT], BF16, tag="xi", name="xi")
            nc.sync.dma_start(
                out=xi, in_=qT_d[640 + 64 * pair:640 + 64 * pair + 64, hsl])
            bro_r = prep.tile([64, HT], BF16, tag="bror", name="bror")
            nc.sync.dma_start(
                out=bro_r, in_=din["broped_r"][64 * pair:64 * pair + 64, hsl])
            bro_i = prep.tile([64, HT], BF16, tag="broi", name="broi")
            nc.sync.dma_start(
                out=bro_i, in_=din["broped_i"][64 * pair:64 * pair + 64, hsl])
            c_, s_ = cosf[:, hsl], sinf[:, hsl]
            t1 = prep.tile([64, HT], BF16, tag="t1", name="t1")
            t2 = prep.tile([64, HT], BF16, tag="t2", name="t2")
            yr = prep.tile([64, HT], BF16, tag="yr", name="yr")
            yi = prep.tile([64, HT], BF16, tag="yi", name="yi")
            nc.vector.tensor_tensor(t1, xr, c_, ALU.mult)
            nc.vector.tensor_tensor(t2, xi, s_, ALU.mult)
            nc.vector.tensor_tensor(yr, t1, t2, ALU.subtract)
            nc.vector.tensor_tensor(t1, xr, s_, ALU.mult)
            nc.vector.tensor_tensor(t2, xi, c_, ALU.mult)
            nc.vector.tensor_tensor(yi, t1, t2, ALU.add)
            nc.vector.tensor_tensor(yr, yr, rq_bc[0:64, hsl], ALU.mult)
            nc.vector.tensor_tensor(yr, yr, bro_r, ALU.add)
            nc.vector.tensor_tensor(yi, yi, rq_bc[0:64, hsl], ALU.mult)
            nc.vector.tensor_tensor(yi, yi, bro_i, ALU.add)
            for i in range(2):
                h = 2 * pair + i
                nc.sync.dma_start(out=qr_f[h][0:32, hsl],
                                  in_=yr[32 * i:32 * i + 32, :])
                nc.sync.dma_start(out=qr_f[h][32:64, hsl],
                                  in_=yi[32 * i:32 * i + 32, :])

        # =================== PHASE I: projections ======================
        with ExitStack() as p1:
            wpool = p1.enter_context(tc.tile_pool(name="w1", bufs=1))
            xpool = p1.enter_context(tc.tile_pool(name="p1x", bufs=2))

            # weights in consumption order, block-granular so each chain
            # starts as its block lands; x0 split so the first chain can
            # begin after ~1.2MB of DMA
            wqa_sb = [wpool.tile([P, KT_D, P], BF16, tag=f"wqa{m}",
                                 name=f"wqa{m}") for m in range(3)]
            nc.sync.dma_start(out=wqa_sb[0], in_=_blk(din["wqaT_sl"], KT_D, P, 0))
            x0_sb = xpool.tile([P, KT_D, CH], BF16, tag="x", name="x0")
            nc.sync.dma_start(
                out=x0_sb[:, 0:8, :],
                in_=bass.AP(tensor=din["xt"].tensor, offset=0,
                            ap=[[T, P], [P * T, 8], [1, CH]]))
            nc.sync.dma_start(
                out=x0_sb[:, 8:KT_D, :],
                in_=bass.AP(tensor=din["xt"].tensor, offset=8 * P * T,
                            ap=[[T, P], [P * T, 8], [1, CH]]))
            for m in range(1, 3):
                nc.sync.dma_start(out=wqa_sb[m],
                                  in_=_blk(din["wqaT_sl"], KT_D, P, P * m))
            wkva_sb = []
            for m in range(5):
                w = 64 if m == 4 else P
                t = wpool.tile([P, KT_D, w], BF16, tag=f"wkva{m}", name=f"wkva{m}")
                nc.sync.dma_start(out=t, in_=_blk(din["wkvaT"], KT_D, w, P * m))
                wkva_sb.append(t)
            wf_sb = []
            for m in range(6):
                t = wpool.tile([P, KT_D, P], BF16, tag=f"wf{m}", name=f"wf{m}")
                nc.sync.dma_start(out=t, in_=_blk(din["wfT"], KT_D, P, P * m))
                wf_sb.append(t)
            wkk_sb = wpool.tile([P, NKV, 512], BF16)
            nc.sync.dma_start(out=wkk_sb, in_=_blk(din["wkvbTk"], NKV, 512))
            wkv_sb = wpool.tile([P, NKV, 512], BF16)
            nc.sync.dma_start(out=wkv_sb, in_=_blk(din["wkvbTv"], NKV, 512))

            cpool = p1.enter_context(tc.tile_pool(name="p1c", bufs=2))
            kvpool = p1.enter_context(tc.tile_pool(name="p1kv", bufs=1))
            mm = p1.enter_context(tc.tile_pool(name="p1ps", bufs=6, space="PSUM"))
            sspool = p1.enter_context(tc.tile_pool(name="p1ss", bufs=2, space="PSUM"))

            def chunk_sweeps(c):
                csl = slice(CH * c, CH * (c + 1))
                if c == 0:
                    x_sb = x0_sb
                else:
                    x_sb = xpool.tile([P, KT_D, CH], BF16, tag="x", name=f"x{c}")
                    nc.sync.dma_start(
                        out=x_sb,
                        in_=bass.AP(tensor=din["xt"].tensor, offset=CH * c,
                                    ap=[[T, P], [P * T, KT_D], [1, CH]]))

                # ---- sweep 1: qmid-slice (3) + kvc (4) + kpe (1) chains ----
                ps_q = [mm.tile([P, 512], F32, tag="mm", name=f"q{m}")
                        for m in range(3)]
                for m in range(3):
                    for k in range(KT_D):
                        nc.tensor.matmul(
                            ps_q[m][:, :CH],
                            wqa_sb[m][:, k, :],
                            x_sb[:, k, :],
                            start=(k == 0), stop=(k == KT_D - 1))
                ps_kv = [mm.tile([P, 512], F32, tag="mm", name=f"kv{m}")
                         for m in range(4)]
                for m in range(4):
                    for k in range(KT_D):
                        nc.tensor.matmul(
                            ps_kv[m][:, :CH],
                            wkva_sb[m][:, k, :],
                            x_sb[:, k, :],
                            start=(k == 0), stop=(k == KT_D - 1))
                ps_kpe = mm.tile([P, 512], F32, tag="mm", name="kpe")
                for k in range(KT_D):
                    nc.tensor.matmul(
                        ps_kpe[:64, :CH],
                        wkva_sb[4][:, k, :],
                        x_sb[:, k, :],
                        start=(k == 0), stop=(k == KT_D - 1))

                # ---- sweep-1 evictions ----
                sq = []   # squared qmid slices (bf16: feeds a bf16 sum-MM)
                for m in range(3):
                    t = cpool.tile([P, CH], BF16, tag=f"sq{m}", name=f"sq{m}", bufs=1)
                    nc.scalar.activation(t, ps_q[m][:, :CH], AF.Square,
                                         bias=bqa_sb[:, m:m + 1])
                    sq.append(t)
                kvc_sb, kvsq = [], []
                for m in range(4):
                    t = kvpool.tile([P, CH], BF16, tag=f"kvc{m}", name=f"kvc{m}",
                                    bufs=4)
                    nc.vector.tensor_scalar(t, ps_kv[m][:, :CH],
                                            bkva_sb[:, m:m + 1], None, ALU.add)
                    kvc_sb.append(t)
                    t2 = cpool.tile([P, CH], BF16, tag=f"kvsq{m}", name=f"kvsq{m}",
                                    bufs=1)
                    nc.scalar.activation(t2, ps_kv[m][:, :CH], AF.Square,
                                         bias=bkva_sb[:, m:m + 1])
                    kvsq.append(t2)
                kpe_raw = cpool.tile([64, CH], F32, tag="kpr", name="kpr", bufs=1)
                nc.scalar.activation(kpe_raw, ps_kpe[:64, :CH], AF.Identity,
                                     bias=bkva_sb[0:64, 4:5])
                # rope-rotate k_pe into kpeT[:, csl]; imag half of the input
                # is DMA-shifted down to partitions 0..31 first
                c32 = cpool.tile([32, CH], BF16, tag="c32", name="c32", bufs=1)
                nc.sync.dma_start(out=c32, in_=din["cosT4"][0:32, csl])
                s32 = cpool.tile([32, CH], BF16, tag="s32", name="s32", bufs=1)
                nc.sync.dma_start(out=s32, in_=din["sinT4"][0:32, csl])
                xr = kpe_raw[0:32, :]
                xi = cpool.tile([32, CH], F32, tag="xikp", name="xikp", bufs=1)
                nc.sync.dma_start(out=xi, in_=kpe_raw[32:64, :])
                t2_ = cpool.tile([32, CH], F32, tag="t2", name="t2", bufs=1)
                yikp = cpool.tile([32, CH], BF16, tag="yikp", name="yikp", bufs=1)
                nc.vector.tensor_tensor(kpeT[0:32, csl], xr, c32, ALU.mult)
                nc.vector.tensor_tensor(t2_, xi, s32, ALU.mult)
                nc.vector.tensor_tensor(kpeT[0:32, csl], kpeT[0:32, csl], t2_,
                                        ALU.subtract)
                nc.vector.tensor_tensor(yikp, xr, s32, ALU.mult)
                nc.vector.tensor_tensor(t2_, xi, c32, ALU.mult)
                nc.vector.tensor_tensor(yikp, yikp, t2_, ALU.add)
                nc.sync.dma_start(out=kpeT[32:64, csl], in_=yikp)

                # ---- sweep 2: fused-q (6 chains) + sumsq chains ----
                ps_fq = [mm.tile([P, 512], F32, tag="mm", name=f"fq{m}")
                         for m in range(6)]
                for m in range(6):
                    for k in range(KT_D):
                        nc.tensor.matmul(
                            ps_fq[m][:, :CH],
                            wf_sb[m][:, k, :],
                            x_sb[:, k, :],
                            start=(k == 0), stop=(k == KT_D - 1))
                ps_qss = sspool.tile([1, 512], F32, tag="ss", name="qss")
                for m in range(3):
                    nc.tensor.matmul(ps_qss[:, :CH], ones_bf, sq[m],
                                     start=(m == 0), stop=(m == 2))
                ps_kss = sspool.tile([1, 512], F32, tag="ss", name="kss")
                for m in range(4):
                    nc.tensor.matmul(ps_kss[:, :CH], ones_bf, kvsq[m],
                                     start=(m == 0), stop=(m == 3))

                # ---- sweep-2 evictions ----
                qch = cpool.tile([P, 6, CH], BF16, tag="qch", name="qch", bufs=1)
                for m in range(6):
                    nc.scalar.activation(qch[:, m, :], ps_fq[m][:, :CH],
                                         AF.Identity, bias=bf_sb[:, m:m + 1])
                nc.sync.dma_start(
                    out=bass.AP(tensor=qT_d.tensor, offset=CH * c,
                                ap=[[T, P], [P * T, 6], [1, CH]]),
                    in_=qch)
                # q-sumsq partial straight to the collective input buffer
                qssv = cpool.tile([1, CH], F32, tag="qssv", name="qssv", bufs=1)
                nc.vector.tensor_copy(qssv, ps_qss[:, :CH])
                nc.gpsimd.dma_start(
                    out=bass.AP(tensor=cc_in.tensor, offset=CH * c,
                                ap=[[CH, 1], [1, CH]]),
                    in_=qssv)
                # rkv for this chunk
                srt = cpool.tile([1, CH], F32, tag="srt", name="srt", bufs=1)
                nc.scalar.activation(srt, ps_kss[:, :CH], AF.Sqrt,
                                     bias=eps_sb[0:1, :], scale=1.0 / KVL)
                rkv_c = cpool.tile([1, CH], F32, tag="rkvc", name="rkvc", bufs=1)
                nc.vector.reciprocal(rkv_c, srt)
                rkv_bc = cpool.tile([P, CH], F32, tag="rkvbc", name="rkvbc",
                                    bufs=4)
                nc.gpsimd.partition_broadcast(rkv_bc, rkv_c)
                for tt in range(2):
                    nc.sync.dma_start(
                        out=rkv_tok[:, 2 * c + tt: 2 * c + tt + 1],
                        in_=rkv_c[:, P * tt:P * (tt + 1)])
                return kvc_sb, rkv_bc

            def chunk_b(c, kvc_sb, rkv_bc):
                ps_kn = [mm.tile([P, 512], F32, tag="mm", name=f"kn{m}")
                         for m in range(4)]
                for m in range(4):
                    for k in range(NKV):
                        nc.tensor.matmul(
                            ps_kn[m][:, :CH],
                            wkk_sb[:, k, P * m:P * (m + 1)],
                            kvc_sb[k],
                            start=(k == 0), stop=(k == NKV - 1))
                ps_v = [mm.tile([P, 512], F32, tag="mm", name=f"v{tt}")
                        for tt in range(2)]
                for tt in range(2):
                    for k in range(NKV):
                        nc.tensor.matmul(
                            ps_v[tt],
                            kvc_sb[k][:, P * tt:P * (tt + 1)],
                            wkv_sb[:, k, :],
                            start=(k == 0), stop=(k == NKV - 1))
                kn_ch = cpool.tile([P, 4, CH], F32R, tag="knch", name="knch",
                                   bufs=2)
                for m in range(4):
                    nc.vector.tensor_tensor(kn_ch[:, m, :], ps_kn[m][:, :CH],
                                            rkv_bc, ALU.mult)
                nc.sync.dma_start(
                    out=bass.AP(tensor=knope_d.tensor, offset=CH * c,
                                ap=[[T, P], [P * T, 4], [1, CH]]),
                    in_=kn_ch)
                v_ch = cpool.tile([P, 2, 512], BF16, tag="vch", name="vch",
                                  bufs=2)
                for tt in range(2):
                    nc.scalar.activation(
                        v_ch[:, tt, :], ps_v[tt], AF.Copy,
                        scale=rkv_tok[:, 2 * c + tt: 2 * c + tt + 1])
                nc.sync.dma_start(
                    out=bass.AP(tensor=v_d.tensor, offset=512 * CH * c,
                                ap=[[512, P], [512 * P, 2], [1, 512]]),
                    in_=v_ch)

            def emit_ag_start(half):
                import os
                cc_out_ap = bass.AP(tensor=cc_out.tensor, offset=half * 4 * HT,
                                    ap=[[HT, 4], [1, HT]])
                if os.environ.get("NO_CC"):
                    for rr in range(4):
                        nc.gpsimd.dma_start(
                            out=bass.AP(tensor=cc_out.tensor,
                                        offset=(half * 4 + rr) * HT,
                                        ap=[[HT, 1], [1, HT]]),
                            in_=cc_in[half:half + 1, :])
                else:
                    nc.gpsimd.collective_compute(
                        "AllGather",
                        ALU.bypass,
                        replica_groups=[[0, 1, 2, 3], [4, 5, 6, 7]],
                        ins=[cc_in[half:half + 1, :]],
                        outs=[cc_out_ap],
                    )

            deferred = {}
            for c in range(NCH):
                kvc_sb, rkv_bc = chunk_sweeps(c)
                if c < DEFER_B:
                    chunk_b(c, kvc_sb, rkv_bc)
                else:
                    deferred[c] = (kvc_sb, rkv_bc)
                if c == 3:
                    emit_ag_start(0)
                if c == 5:
                    emit_ag_finish(0, use_sync_dma=False)
            emit_ag_start(1)
            # deferred B-work lands in the PE queue right where phase II
            # would otherwise starve waiting for q-prep; the pair-0 half-0
            # q prep (loads + DVE) interleaves with it
            cosf = apool.tile([64, T], BF16)
            nc.sync.dma_start(out=cosf, in_=din["cosT4"][0:64, :])
            sinf = apool.tile([64, T], BF16)
            nc.sync.dma_start(out=sinf, in_=din["sinT4"][0:64, :])
            qn_f, qr_f = prep_alloc(qpool, 0)
            prep_pair_half(0, 0, qn_f, qr_f, cosf, sinf)
            for c in sorted(deferred):
                chunk_b(c, *deferred[c])

        # =================== PHASE II: q prep + attention + out ==========
        with ExitStack() as p2:
            qpool2 = p2.enter_context(tc.tile_pool(name="p2q2", bufs=1))
            hpool = p2.enter_context(tc.tile_pool(name="p2h", bufs=2))
            cpool = p2.enter_context(tc.tile_pool(name="p2c", bufs=2))
            espool = p2.enter_context(tc.tile_pool(name="p2e", bufs=2))
            opool = p2.enter_context(tc.tile_pool(name="p2osb", bufs=16))
            oute = p2.enter_context(tc.tile_pool(name="p2oute", bufs=2))
            p2w = p2.enter_context(tc.tile_pool(name="p2w", bufs=1))
            mm2 = p2.enter_context(tc.tile_pool(name="p2s", bufs=4, space="PSUM"))
            omm = p2.enter_context(tc.tile_pool(name="p2o", bufs=2, space="PSUM"))
            p4mm = p2.enter_context(tc.tile_pool(name="p2p4", bufs=2, space="PSUM"))

            def load_head(h, full):
                # uniform tile sizes; block-0 loads fill only the first half
                cols = T if full else HT
                nkt = TT if full else TT // 2
                kn = hpool.tile([P, T], F32R, tag="kn", name=f"kn{h}{full}")
                nc.sync.dma_start(out=kn[:, 0:cols],
                                  in_=knope_d[P * h:P * (h + 1), 0:cols])
                vh = hpool.tile([P, TT, P], BF16, tag="vh", name=f"vh{h}{full}")
                nc.sync.dma_start(
                    out=vh[:, 0:nkt, :],
                    in_=bass.AP(tensor=v_d.tensor, offset=P * h,
                                ap=[[512, P], [P * 512, nkt], [1, P]]))
                return kn, vh

            o_sb = {}

            def emit_p4(qch):
                qsl = slice(512 * qch, 512 * (qch + 1))
                for m in range(DIM // P):
                    ps = p4mm.tile([P, 512], F32, tag="p4", name="p4")
                    for hh in range(4):
                        nc.tensor.matmul(
                            ps, wo_sb[:, hh, P * m:P * (m + 1)],
                            o_sb[(hh, qch)],
                            start=(hh == 0), stop=(hh == 3))
                    ot = oute.tile([P, 512], F32, tag="ot", name="ot")
                    if m % 2 == 0:
                        nc.scalar.activation(ot, ps, AF.Copy)
                    else:
                        nc.vector.tensor_copy(ot, ps)
                    nc.sync.dma_start(out=outT[P * m:P * (m + 1), qsl], in_=ot)

            def attention_qch(h, kn, vh, qn_f, qr_f, qch, with_p4):
                qsl = slice(512 * qch, 512 * (qch + 1))
                n_kt = 4 * (qch + 1)
                es = espool.tile([P, TT, 512], BF16, tag="es", name="es")
                for kt in range(n_kt):
                    ps = mm2.tile([P, 512], F32, tag="s", name="s")
                    nc.tensor.matmul(ps, r32(kn[:, P * kt:P * (kt + 1)]),
                                     r32(qn_f[h][:, qsl]),
                                     start=True, stop=False)
                    nc.tensor.matmul(ps, kpeT[:, P * kt:P * (kt + 1)],
                                     qr_f[h][:, qsl],
                                     start=False, stop=True)
                    di = kt - 4 * qch
                    if di >= 0:
                        nc.vector.tensor_tensor(ps, ps, masks_sb[:, di, :],
                                                ALU.add)
                    nc.scalar.activation(es[:, kt, :], ps, AF.Exp,
                                         scale=SCALE)
                o_ps = omm.tile([P, 512], F32, tag="o", name="o")
                for kt in range(n_kt):
                    nc.tensor.matmul(o_ps, vh[:, kt, :], es[:, kt, :],
                                     start=(kt == 0), stop=(kt == n_kt - 1))
                # softmax denominator off the PE: bf16 tree-sum over the kt
                # tiles, then a cross-partition reduce on GPSIMD
                ses = cpool.tile([P, 512], BF16, tag="ses", name="ses")
                nc.vector.tensor_tensor(ses, es[:, 0, :], es[:, 1, :], ALU.add)
                for kt in range(2, n_kt):
                    nc.vector.tensor_tensor(ses, ses, es[:, kt, :], ALU.add)
                sesum = cpool.tile([P, 512], F32, tag="sesum", name="sesum")
                nc.gpsimd.partition_all_reduce(sesum, ses, channels=P,
                                               reduce_op=bass_isa.ReduceOp.add)
                rec = cpool.tile([P, 512], F32, tag="rec", name="rec")
                nc.vector.reciprocal(rec, sesum)
                och = opool.tile([P, 512], BF16, tag="och", name=f"o{h}{qch}")
                nc.vector.tensor_tensor(och, o_ps, rec, ALU.mult)
                nc.vector.tensor_scalar(och, och, bv_sb[:, h:h + 1], None,
                                        ALU.add)
                o_sb[(h, qch)] = och
                if with_p4:
                    emit_p4(qch)

            # ---- query-block 0 (qch 0,1): needs only rq half 0 ----
            kn0, vh0 = load_head(0, False)
            masks_sb = p2w.tile([P, 4, 512], F32)
            nc.sync.dma_start(
                out=masks_sb,
                in_=bass.AP(tensor=din["masks01"].tensor, offset=0,
                            ap=[[512, P], [P * 512, 4], [1, 512]]))
            attention_qch(0, kn0, vh0, qn_f, qr_f, 0, False)
            attention_qch(0, kn0, vh0, qn_f, qr_f, 1, False)
            qn_f2, qr_f2 = prep_alloc(qpool2, 1)
            prep_pair_half(1, 0, qn_f2, qr_f2, cosf, sinf)
            kn1, vh1 = load_head(1, False)
            attention_qch(1, kn1, vh1, qn_f, qr_f, 0, False)
            attention_qch(1, kn1, vh1, qn_f, qr_f, 1, False)
            wo_sb = p2w.tile([P, 4, T], BF16)
            nc.sync.dma_start(out=wo_sb, in_=_blk(din["woT"], 4, T))
            kn2, vh2 = load_head(2, False)
            attention_qch(2, kn2, vh2, qn_f2, qr_f2, 0, False)
            attention_qch(2, kn2, vh2, qn_f2, qr_f2, 1, False)
            kn3, vh3 = load_head(3, False)
            attention_qch(3, kn3, vh3, qn_f2, qr_f2, 0, True)
            attention_qch(3, kn3, vh3, qn_f2, qr_f2, 1, True)

            # ---- second-half rq: AllGather #2 results, then half-1 q prep;
            # all of this hides under block-0 attention ----
            emit_ag_finish(1, use_sync_dma=False)
            prep_pair_half(0, 1, qn_f, qr_f, cosf, sinf)
            prep_pair_half(1, 1, qn_f2, qr_f2, cosf, sinf)

            # ---- query-block 1 (qch 2,3) ----
            kn0b, vh0b = load_head(0, True)
            attention_qch(0, kn0b, vh0b, qn_f, qr_f, 2, False)
            attention_qch(0, kn0b, vh0b, qn_f, qr_f, 3, False)
            kn1b, vh1b = load_head(1, True)
            attention_qch(1, kn1b, vh1b, qn_f, qr_f, 2, False)
            attention_qch(1, kn1b, vh1b, qn_f, qr_f, 3, False)
            kn2b, vh2b = load_head(2, True)
            attention_qch(2, kn2b, vh2b, qn_f2, qr_f2, 2, False)
            attention_qch(2, kn2b, vh2b, qn_f2, qr_f2, 3, False)
            kn3b, vh3b = load_head(3, True)
            attention_qch(3, kn3b, vh3b, qn_f2, qr_f2, 2, True)
            attention_qch(3, kn3b, vh3b, qn_f2, qr_f2, 3, True)

            if "kpe_o" in scratch:
                nc.sync.dma_start(out=scratch["kpe_o"], in_=kpeT.bitcast(F32))
                nc.sync.dma_start(out=scratch["rq_o"], in_=rq_bc)


# ---------------------------------------------------------------- entry

_NC_CACHE = {}


def _get_nc():
    if "nc" not in _NC_CACHE:
        _NC_CACHE["nc"] = build_bass()
    return _NC_CACHE["nc"]


def _run(inputs, trace=False):
    cores = _host_prep(inputs)
    nc = _get_nc()
    in_maps = [{k: d[k] for k in INPUT_SPECS} for d in cores]
    res = run_bass_kernel_spmd(nc, in_maps, core_ids=list(range(8)), trace=trace)
    outs = [res.results[c]["outT"] for c in range(8)]
    final = np.zeros((B, S, DIM), np.float32)
    wo_b = np.asarray(inputs["wo_b"], np.float32)
    for b in range(B):
        acc = outs[4 * b].copy()
        for g in range(1, HG):
            acc += outs[4 * b + g]
        final[b] = acc.T + wo_b[None, :]
    return final, res


def kernel(**inputs):
    return _run(inputs, trace=False)[0]


def kernel_profiled(**inputs):
    return _run(inputs, trace=False)


# revision 43
# speedup vs baseline: 1.5655x; 1.0388x over previous
"""MLA-style attention (DeepSeek MLA block) on 8 Trainium2 NeuronCores.

Sharding: core c = b*4 + g  (batch b in {0,1}, head-group g in {0..3} = 4 heads).

Key structure (v3):
- The q low-rank path is ABSORBED on the host: Wf = (wq_b * q_norm)[group rows]
  @ wq_a  (768 x 2048). Since the rmsnorm per-token scale r_t commutes through
  the second projection, q = r_t * (Wf x + bf) + b_qb. Each core computes only
  a 384-row slice of wq_a x for the sum-of-squares that defines r_t; the four
  cores of a batch AllGather their 8KB partials (2 collectives, pipelined).
  Collective bounce DMAs must go through gpsimd/SWDGE (HWDGE transfers
  adjacent to a collective get chopped).
- kv path stays two-stage (contraction 512 beats 2048) with kvc replicated.
- k-bias is dropped entirely (softmax is invariant to per-query score shifts);
  v-bias is added after the softmax (rows sum to 1); q biases are applied
  during the on-chip q prep (rope rotation + r_t scaling).
- Inputs x and all first-stage weights are bf16 (same PE throughput as fp32r,
  fp32 accumulation); scores/q/k SBUF tiles stay fp32r. The value path
  (exp(scores), v, attention output, wo) is bf16.
- Softmax denominator via bf16 tree-add + GPSIMD cross-partition reduce
  (keeps the PE free; frees a PSUM bank for score pipelining).
- B-stage for the last chunks is deferred into the phase-II transition to
  keep the PE busy while q-prep loads/rotates.
"""
import ml_dtypes
import numpy as np

import concourse.bass as bass
import concourse.tile as tile
from concourse import bacc, bass_isa, mybir
from concourse.bass_utils import run_bass_kernel_spmd

F32 = mybir.dt.float32
F32R = mybir.dt.float32r
BF16 = mybir.dt.bfloat16
BF_NP = ml_dtypes.bfloat16

B, S, DIM = 2, 2048, 2048
NH = 16
QL, KVL = 1536, 512
NOPE, ROPE, VHD = 128, 64, 128
QK_HD = NOPE + ROPE
EPS = 1e-6
SCALE = QK_HD ** -0.5
HG = 4            # heads per group
T = S
P = 128
KT_D = DIM // P   # 16 contraction tiles over model dim
CH = 256          # phase-I token chunk
NCH = T // CH     # 8 chunks
QSL = QL // HG    # 384 qmid slice rows per core
NKV = 4           # kvc feature tiles (512)
TT = T // P       # 16 token tiles
HT = T // 2
DEFER_B = 4       # chunks >= this get their B-stage after the last sweep


def r32(ap):
    return ap.bitcast(F32R)


# ---------------------------------------------------------------- host side

def _host_prep(inp):
    cos = np.asarray(inp["freqs_cos"], np.float32)   # (S, 32)
    sin = np.asarray(inp["freqs_sin"], np.float32)
    cosT4 = np.ascontiguousarray(np.tile(cos.T, (4, 1))).astype(BF_NP)
    sinT4 = np.ascontiguousarray(np.tile(sin.T, (4, 1))).astype(BF_NP)

    wqa = np.asarray(inp["wq_a_w"], np.float32)          # (1536, 2048)
    bqa = np.asarray(inp["wq_a_b"], np.float32)
    qn = np.asarray(inp["q_norm_w"], np.float32)
    wqb = np.asarray(inp["wq_b_w"], np.float32) * qn[None, :]   # (3072, 1536)
    bqb = np.asarray(inp["wq_b_b"], np.float32)

    perm_kva = np.concatenate([
        np.arange(KVL),
        KVL + 2 * np.arange(32),
        KVL + 2 * np.arange(32) + 1,
    ])
    wkvaT = np.ascontiguousarray(
        np.asarray(inp["wkv_a_w"], np.float32)[perm_kva].T).astype(BF_NP)
    bkva_p = np.asarray(inp["wkv_a_b"], np.float32)[perm_kva]
    bkva = np.zeros((5, P), np.float32)
    bkva.reshape(-1)[:576] = bkva_p

    kvn = np.asarray(inp["kv_norm_w"], np.float32)
    wkvb = np.asarray(inp["wkv_b_w"], np.float32) * kvn[None, :]
    bkvb = np.asarray(inp["wkv_b_b"], np.float32)
    wo = np.asarray(inp["wo_w"], np.float32)

    # additive causal masks per diagonal sub-tile: -1e30 where (128*di+k) > q
    masks01 = np.zeros((4, P, 512), np.float32)
    for di in range(4):
        kk = np.arange(P)[:, None] + P * di
        qq = np.arange(512)[None, :]
        masks01[di] = np.where(kk > qq, -1e30, 0.0).astype(np.float32)

    cores = []
    for b in range(B):
        xt = np.ascontiguousarray(
            np.asarray(inp["x"], np.float32)[b].T).astype(BF_NP)
        for g in range(HG):
            heads = range(4 * g, 4 * g + 4)
            rows_nope = np.concatenate(
                [np.arange(h * QK_HD, h * QK_HD + NOPE) for h in heads])
            rows_real = np.concatenate(
                [h * QK_HD + NOPE + 2 * np.arange(32) for h in heads])
            rows_imag = np.concatenate(
                [h * QK_HD + NOPE + 2 * np.arange(32) + 1 for h in heads])
            rows_q = np.concatenate([rows_nope, rows_real, rows_imag])

            wf = wqb[rows_q] @ wqa                     # (768, 2048)
            bf = wqb[rows_q] @ bqa                     # (768,)
            bqb_n = bqb[rows_nope]                     # (512,)
            br, bi = bqb[rows_real], bqb[rows_imag]    # (128,) each
            fidx = np.tile(np.arange(32), 4)
            broped_r = (br[:, None] * cos.T[fidx] - bi[:, None] * sin.T[fidx])
            broped_i = (br[:, None] * sin.T[fidx] + bi[:, None] * cos.T[fidx])

            rows_k = np.concatenate(
                [np.arange(h * (NOPE + VHD), h * (NOPE + VHD) + NOPE) for h in heads])
            rows_v = np.concatenate(
                [np.arange(h * (NOPE + VHD) + NOPE, (h + 1) * (NOPE + VHD)) for h in heads])

            # packed per-feature scalars, cols: 0:3 bqa_slice | 3:8 bkva |
            # 8:14 bfused | 14:18 bqb_nope | 18:22 bv | 22 eps
            biases = np.zeros((32, P), np.float32)
            biases[0:3] = bqa[QSL * g: QSL * (g + 1)].reshape(3, P)
            biases[3:8] = bkva
            biases[8:14] = bf.reshape(6, P)
            biases[14:18] = bqb_n.reshape(4, P)
            biases[18:22] = bkvb[rows_v].reshape(4, P)
            biases[22] = EPS
            cores.append(dict(
                xt=xt,
                wqaT_sl=np.ascontiguousarray(
                    wqa[QSL * g: QSL * (g + 1)].T).astype(BF_NP),
                wkvaT=wkvaT,
                wfT=np.ascontiguousarray(wf.T).astype(BF_NP),
                wkvbTk=np.ascontiguousarray(wkvb[rows_k].T).astype(BF_NP),
                wkvbTv=np.ascontiguousarray(wkvb[rows_v].T).astype(BF_NP),
                woT=np.ascontiguousarray(
                    wo[:, 512 * g: 512 * (g + 1)].T).astype(BF_NP),
                biases=biases,
                cosT4=cosT4, sinT4=sinT4,
                broped_r=np.ascontiguousarray(broped_r).astype(BF_NP),
                broped_i=np.ascontiguousarray(broped_i).astype(BF_NP),
                masks01=masks01,
                ones_bf=np.ones((P, 1), BF_NP),
            ))
    return cores


INPUT_SPECS = dict(
    xt=((DIM, T), BF16),
    wqaT_sl=((DIM, QSL), BF16),
    wkvaT=((DIM, 576), BF16),
    wfT=((DIM, 768), BF16),
    wkvbTk=((KVL, 512), BF16),
    wkvbTv=((KVL, 512), BF16),
    woT=((512, DIM), BF16),
    ones_bf=((P, 1), BF16),
    biases=((32, P), F32),
    cosT4=((P, T), BF16), sinT4=((P, T), BF16),
    broped_r=((P, T), BF16), broped_i=((P, T), BF16),
    masks01=((4, P, 512), F32),
)


# ---------------------------------------------------------------- device IR

def _blk(w, kt, width, m0=0):
    """AP over DRAM weight w (R, C): (128p rows, kt ktiles, width cols at m0)."""
    rows, cols = w.shape
    return bass.AP(
        tensor=w.tensor, offset=m0,
        ap=[[cols, P], [P * cols, kt], [1, width]],
    )


def _t_view(a2d):
    arows, acols = a2d.shape
    assert acols == P
    return bass.AP(tensor=a2d.tensor, offset=0, ap=[[1, P], [P, arows]])


def build_bass(debug_taps=False):
    nc = bacc.Bacc("TRN2", target_bir_lowering=False, debug=False, num_devices=8)

    din = {name: nc.dram_tensor(name, shape, dt, kind="ExternalInput").ap()
           for name, (shape, dt) in INPUT_SPECS.items()}
    outT = nc.dram_tensor("outT", (DIM, T), F32, kind="ExternalOutput").ap()
    kind = "ExternalOutput" if debug_taps else "Internal"
    scratch = dict(
        qT_d=nc.dram_tensor("qT_d", (768, T), BF16, kind=kind).ap(),
        knope_d=nc.dram_tensor("knope_d", (512, T), F32R, kind=kind).ap(),
        v_d=nc.dram_tensor("v_d", (T, 512), BF16, kind=kind).ap(),
        cc_in=nc.dram_tensor("cc_in", (2, HT), F32).ap(),
        cc_out=nc.dram_tensor("cc_out", (2, 4, HT), F32).ap(),
    )
    if debug_taps:
        scratch["kpe_o"] = nc.dram_tensor("kpe_o", (64, T), F32, kind=kind).ap()
        scratch["rq_o"] = nc.dram_tensor("rq_o", (P, T), F32, kind=kind).ap()

    with tile.TileContext(nc) as tc:
        _emit(tc, din, outT, scratch)

    nc.compile()
    return nc


def _emit(tc, din, outT, scratch):
    nc = tc.nc
    from contextlib import ExitStack
    ALU = mybir.AluOpType
    AF = mybir.ActivationFunctionType
    qT_d, knope_d, v_d = scratch["qT_d"], scratch["knope_d"], scratch["v_d"]
    cc_in, cc_out = scratch["cc_in"], scratch["cc_out"]

    with ExitStack() as outer:
        const = outer.enter_context(tc.tile_pool(name="const", bufs=1))
        ones_bf = const.tile([P, 1], BF16)
        nc.sync.dma_start(out=ones_bf, in_=din["ones_bf"])
        bs = const.tile([P, 32], F32)
        nc.sync.dma_start(out=bs, in_=_t_view(din["biases"]))
        bqa_sb, bkva_sb = bs[:, 0:3], bs[:, 3:8]
        bf_sb, bqbn_sb, bv_sb = bs[:, 8:14], bs[:, 14:18], bs[:, 18:22]
        eps_sb = bs[:, 22:23]
        kpeT = const.tile([64, T], BF16)      # roped shared k_pe [real|imag]
        rkv_tok = const.tile([P, TT], F32)
        rq_bc = const.tile([P, T], F32)       # broadcast 1/rms(q) per token
        apool = outer.enter_context(tc.tile_pool(name="p2a", bufs=1))
        qpool = outer.enter_context(tc.tile_pool(name="p2q", bufs=1))
        prep = outer.enter_context(tc.tile_pool(name="p2p", bufs=1))
        agp = outer.enter_context(tc.tile_pool(name="agp", bufs=1))
        cc_out_t = scratch["cc_out"]

        def emit_ag_finish(half, use_sync_dma):
            AF = mybir.ActivationFunctionType
            for qc in range(4):
                off = half * 4 * HT + CH * qc
                gath = agp.tile([4, CH], F32, tag="gath", name="gath", bufs=1)
                dma = nc.sync.dma_start if use_sync_dma else nc.gpsimd.dma_start
                dma(out=gath,
                    in_=bass.AP(tensor=cc_out_t.tensor, offset=off,
                                ap=[[HT, 4], [1, CH]]))
                asum = agp.tile([4, CH], F32, tag="asum", name="asum", bufs=1)
                nc.gpsimd.partition_all_reduce(
                    asum, gath, channels=4, reduce_op=bass_isa.ReduceOp.add)
                srt = agp.tile([1, CH], F32, tag="qsrt", name="qsrt", bufs=1)
                nc.scalar.activation(srt, asum[0:1, :], AF.Sqrt,
                                     bias=eps_sb[0:1, :], scale=1.0 / QL)
                rqh = agp.tile([1, CH], F32, tag="rqh", name="rqh", bufs=1)
                nc.vector.reciprocal(rqh, srt)
                gsl = slice(HT * half + CH * qc, HT * half + CH * (qc + 1))
                nc.gpsimd.partition_broadcast(rq_bc[:, gsl], rqh)

        def prep_alloc(pool, pair):
            qn_f, qr_f = {}, {}
            for i in range(2):
                h = 2 * pair + i
                qn_f[h] = pool.tile([P, T], F32R, tag=f"qn{i}", name=f"qnf{h}")
                qr_f[h] = pool.tile([64, T], BF16, tag=f"qr{i}", name=f"qrf{h}")
            return qn_f, qr_f

        def prep_pair_half(pair, half, qn_f, qr_f, cosf, sinf):
            ALU = mybir.AluOpType
            qT_d = scratch["qT_d"]
            hsl = slice(HT * half, HT * (half + 1))
            for i in range(2):
                h = 2 * pair + i
                tmp = prep.tile([P, HT], BF16, tag="qtmp", name="qtmp")
                nc.sync.dma_start(out=tmp, in_=qT_d[P * h:P * (h + 1), hsl])
                nc.vector.tensor_tensor(qn_f[h][:, hsl], tmp,
                                        rq_bc[:, hsl], ALU.mult)
                nc.vector.tensor_scalar(
                    qn_f[h][:, hsl], qn_f[h][:, hsl],
                    bqbn_sb[:, h:h + 1], None, ALU.add)
            xr = prep.tile([64, HT], BF16, tag="xr", name="xr")
            nc.sync.dma_start(
                out=xr, in_=qT_d[512 + 64 * pair:512 + 64 * pair + 64, hsl])
            xi = prep.tile([64, HT], BF16, tag="xi", name="xi")
            nc.sync.dma_start(
                out=xi, in_=qT_d[640 + 64 * pair:640 + 64 * pair + 64, hsl])
            bro_r = prep.tile([64, HT], BF16, tag="bror", name="bror")
            nc.sync.dma_start(
                out=bro_r, in_=din["broped_r"][64 * pair:64 * pair + 64, hsl])
            bro_i = prep.tile([64, HT], BF16, tag="broi", name="broi")
            nc.sync.dma_start(
                out=bro_i, in_=din["broped_i"][64 * pair:64 * pair + 64, hsl])
            c_, s_ = cosf[:, hsl], sinf[:, hsl]
            t1 = prep.tile([64, HT], BF16, tag="t1", name="t1")
            t2 = prep.tile([64, HT], BF16, tag="t2", name="t2")
            yr = prep.tile([64, HT], BF16, tag="yr", name="yr")
            yi = prep.tile([64, HT], BF16, tag="yi", name="yi")
            nc.vector.tensor_tensor(t1, xr, c_, ALU.mult)
            nc.vector.tensor_tensor(t2, xi, s_, ALU.mult)
            nc.vector.tensor_tensor(yr, t1, t2, ALU.subtract)
            nc.vector.tensor_tensor(t1, xr, s_, ALU.mult)
            nc.vector.tensor_tensor(t2, xi, c_, ALU.mult)
            nc.vector.tensor_tensor(yi, t1, t2, ALU.add)
            nc.vector.tensor_tensor(yr, yr, rq_bc[0:64, hsl], ALU.mult)
            nc.vector.tensor_tensor(yr, yr, bro_r, ALU.add)
            nc.vector.tensor_tensor(yi, yi, rq_bc[0:64, hsl], ALU.mult)
            nc.vector.tensor_tensor(yi, yi, bro_i, ALU.add)
            for i in range(2):
                h = 2 * pair + i
                nc.sync.dma_start(out=qr_f[h][0:32, hsl],
                                  in_=yr[32 * i:32 * i + 32, :])
                nc.sync.dma_start(out=qr_f[h][32:64, hsl],
                                  in_=yi[32 * i:32 * i + 32, :])

        # =================== PHASE I: projections ======================
        with ExitStack() as p1:
            wpool = p1.enter_context(tc.tile_pool(name="w1", bufs=1))
            xpool = p1.enter_context(tc.tile_pool(name="p1x", bufs=2))

            # weights in consumption order, block-granular so each chain
            # starts as its block lands; x0 split so the first chain can
            # begin after ~1.2MB of DMA
            wqa_sb = [wpool.tile([P, KT_D, P], BF16, tag=f"wqa{m}",
                                 name=f"wqa{m}") for m in range(3)]
            nc.sync.dma_start(out=wqa_sb[0], in_=_blk(din["wqaT_sl"], KT_D, P, 0))
            x0_sb = xpool.tile([P, KT_D, CH], BF16, tag="x", name="x0")
            nc.sync.dma_start(
                out=x0_sb[:, 0:8, :],
                in_=bass.AP(tensor=din["xt"].tensor, offset=0,
                            ap=[[T, P], [P * T, 8], [1, CH]]))
            nc.sync.dma_start(
                out=x0_sb[:, 8:KT_D, :],
                in_=bass.AP(tensor=din["xt"].tensor, offset=8 * P * T,
                            ap=[[T, P], [P * T, 8], [1, CH]]))
            for m in range(1, 3):
                nc.sync.dma_start(out=wqa_sb[m],
                                  in_=_blk(din["wqaT_sl"], KT_D, P, P * m))
            wkva_sb = []
            for m in range(5):
                w = 64 if m == 4 else P
                t = wpool.tile([P, KT_D, w], BF16, tag=f"wkva{m}", name=f"wkva{m}")
                nc.sync.dma_start(out=t, in_=_blk(din["wkvaT"], KT_D, w, P * m))
                wkva_sb.append(t)
            wf_sb = []
            for m in range(6):
                t = wpool.tile([P, KT_D, P], BF16, tag=f"wf{m}", name=f"wf{m}")
                nc.sync.dma_start(out=t, in_=_blk(din["wfT"], KT_D, P, P * m))
                wf_sb.append(t)
            wkk_sb = wpool.tile([P, NKV, 512], BF16)
            nc.sync.dma_start(out=wkk_sb, in_=_blk(din["wkvbTk"], NKV, 512))
            wkv_sb = wpool.tile([P, NKV, 512], BF16)
            nc.sync.dma_start(out=wkv_sb, in_=_blk(din["wkvbTv"], NKV, 512))

            cpool = p1.enter_context(tc.tile_pool(name="p1c", bufs=2))
            kvpool = p1.enter_context(tc.tile_pool(name="p1kv", bufs=1))
            mm = p1.enter_context(tc.tile_pool(name="p1ps", bufs=6, space="PSUM"))
            sspool = p1.enter_context(tc.tile_pool(name="p1ss", bufs=2, space="PSUM"))

            x_tiles = {0: x0_sb}

            def prefetch_x(c):
                if c < NCH and c not in x_tiles:
                    t = xpool.tile([P, KT_D, CH], BF16, tag="x", name=f"x{c}")
                    nc.sync.dma_start(
                        out=t,
                        in_=bass.AP(tensor=din["xt"].tensor, offset=CH * c,
                                    ap=[[T, P], [P * T, KT_D], [1, CH]]))
                    x_tiles[c] = t

            def chunk_sweeps(c):
                csl = slice(CH * c, CH * (c + 1))
                prefetch_x(c)
                x_sb = x_tiles.pop(c)

                # ---- sweep 1: qmid-slice (3) + kvc (4) + kpe (1) chains ----
                ps_q = [mm.tile([P, 512], F32, tag="mm", name=f"q{m}")
                        for m in range(3)]
                for m in range(3):
                    for k in range(KT_D):
                        nc.tensor.matmul(
                            ps_q[m][:, :CH],
                            wqa_sb[m][:, k, :],
                            x_sb[:, k, :],
                            start=(k == 0), stop=(k == KT_D - 1))
                ps_kv = [mm.tile([P, 512], F32, tag="mm", name=f"kv{m}")
                         for m in range(4)]
                for m in range(4):
                    for k in range(KT_D):
                        nc.tensor.matmul(
                            ps_kv[m][:, :CH],
                            wkva_sb[m][:, k, :],
                            x_sb[:, k, :],
                            start=(k == 0), stop=(k == KT_D - 1))
                ps_kpe = mm.tile([P, 512], F32, tag="mm", name="kpe")
                for k in range(KT_D):
                    nc.tensor.matmul(
                        ps_kpe[:64, :CH],
                        wkva_sb[4][:, k, :],
                        x_sb[:, k, :],
                        start=(k == 0), stop=(k == KT_D - 1))

                prefetch_x(c + 1)

                # ---- sweep-1 evictions ----
                sq = []   # squared qmid slices (bf16: feeds a bf16 sum-MM)
                for m in range(3):
                    t = cpool.tile([P, CH], BF16, tag=f"sq{m}", name=f"sq{m}", bufs=1)
                    nc.scalar.activation(t, ps_q[m][:, :CH], AF.Square,
                                         bias=bqa_sb[:, m:m + 1])
                    sq.append(t)
                kvc_sb, kvsq = [], []
                for m in range(4):
                    t = kvpool.tile([P, CH], BF16, tag=f"kvc{m}", name=f"kvc{m}",
                                    bufs=4)
                    nc.vector.tensor_scalar(t, ps_kv[m][:, :CH],
                                            bkva_sb[:, m:m + 1], None, ALU.add)
                    kvc_sb.append(t)
                    t2 = cpool.tile([P, CH], BF16, tag=f"kvsq{m}", name=f"kvsq{m}",
                                    bufs=1)
                    nc.scalar.activation(t2, ps_kv[m][:, :CH], AF.Square,
                                         bias=bkva_sb[:, m:m + 1])
                    kvsq.append(t2)
                kpe_raw = cpool.tile([64, CH], F32, tag="kpr", name="kpr", bufs=1)
                nc.scalar.activation(kpe_raw, ps_kpe[:64, :CH], AF.Identity,
                                     bias=bkva_sb[0:64, 4:5])
                # rope-rotate k_pe into kpeT[:, csl]; imag half of the input
                # is DMA-shifted down to partitions 0..31 first
                c32 = cpool.tile([32, CH], BF16, tag="c32", name="c32", bufs=1)
                nc.sync.dma_start(out=c32, in_=din["cosT4"][0:32, csl])
                s32 = cpool.tile([32, CH], BF16, tag="s32", name="s32", bufs=1)
                nc.sync.dma_start(out=s32, in_=din["sinT4"][0:32, csl])
                xr = kpe_raw[0:32, :]
                xi = cpool.tile([32, CH], F32, tag="xikp", name="xikp", bufs=1)
                nc.sync.dma_start(out=xi, in_=kpe_raw[32:64, :])
                t2_ = cpool.tile([32, CH], F32, tag="t2", name="t2", bufs=1)
                yikp = cpool.tile([32, CH], BF16, tag="yikp", name="yikp", bufs=1)
                nc.vector.tensor_tensor(kpeT[0:32, csl], xr, c32, ALU.mult)
                nc.vector.tensor_tensor(t2_, xi, s32, ALU.mult)
                nc.vector.tensor_tensor(kpeT[0:32, csl], kpeT[0:32, csl], t2_,
                                        ALU.subtract)
                nc.vector.tensor_tensor(yikp, xr, s32, ALU.mult)
                nc.vector.tensor_tensor(t2_, xi, c32, ALU.mult)
                nc.vector.tensor_tensor(yikp, yikp, t2_, ALU.add)
                nc.sync.dma_start(out=kpeT[32:64, csl], in_=yikp)

                # ---- sweep 2: fused-q (6 chains) + sumsq chains ----
                ps_fq = [mm.tile([P, 512], F32, tag="mm", name=f"fq{m}")
                         for m in range(6)]
                for m in range(6):
                    for k in range(KT_D):
                        nc.tensor.matmul(
                            ps_fq[m][:, :CH],
                            wf_sb[m][:, k, :],
                            x_sb[:, k, :],
                            start=(k == 0), stop=(k == KT_D - 1))
                ps_qss = sspool.tile([1, 512], F32, tag="ss", name="qss")
                for m in range(3):
                    nc.tensor.matmul(ps_qss[:, :CH], ones_bf, sq[m],
                                     start=(m == 0), stop=(m == 2))
                ps_kss = sspool.tile([1, 512], F32, tag="ss", name="kss")
                for m in range(4):
                    nc.tensor.matmul(ps_kss[:, :CH], ones_bf, kvsq[m],
                                     start=(m == 0), stop=(m == 3))

                # ---- sweep-2 evictions ----
                qch = cpool.tile([P, 6, CH], BF16, tag="qch", name="qch", bufs=1)
                for m in range(6):
                    nc.scalar.activation(qch[:, m, :], ps_fq[m][:, :CH],
                                         AF.Identity, bias=bf_sb[:, m:m + 1])
                nc.sync.dma_start(
                    out=bass.AP(tensor=qT_d.tensor, offset=CH * c,
                                ap=[[T, P], [P * T, 6], [1, CH]]),
                    in_=qch)
                # q-sumsq partial straight to the collective input buffer
                qssv = cpool.tile([1, CH], F32, tag="qssv", name="qssv", bufs=1)
                nc.vector.tensor_copy(qssv, ps_qss[:, :CH])
                nc.gpsimd.dma_start(
                    out=bass.AP(tensor=cc_in.tensor, offset=CH * c,
                                ap=[[CH, 1], [1, CH]]),
                    in_=qssv)
                # rkv for this chunk
                srt = cpool.tile([1, CH], F32, tag="srt", name="srt", bufs=1)
                nc.scalar.activation(srt, ps_kss[:, :CH], AF.Sqrt,
                                     bias=eps_sb[0:1, :], scale=1.0 / KVL)
                rkv_c = cpool.tile([1, CH], F32, tag="rkvc", name="rkvc", bufs=1)
                nc.vector.reciprocal(rkv_c, srt)
                rkv_bc = cpool.tile([P, CH], F32, tag="rkvbc", name="rkvbc",
                                    bufs=4)
                nc.gpsimd.partition_broadcast(rkv_bc, rkv_c)
                for tt in range(2):
                    nc.sync.dma_start(
                        out=rkv_tok[:, 2 * c + tt: 2 * c + tt + 1],
                        in_=rkv_c[:, P * tt:P * (tt + 1)])
                return kvc_sb, rkv_bc

            def chunk_b(c, kvc_sb, rkv_bc):
                ps_kn = [mm.tile([P, 512], F32, tag="mm", name=f"kn{m}")
                         for m in range(4)]
                for m in range(4):
                    for k in range(NKV):
                        nc.tensor.matmul(
                            ps_kn[m][:, :CH],
                            wkk_sb[:, k, P * m:P * (m + 1)],
                            kvc_sb[k],
                            start=(k == 0), stop=(k == NKV - 1))
                ps_v = [mm.tile([P, 512], F32, tag="mm", name=f"v{tt}")
                        for tt in range(2)]
                for tt in range(2):
                    for k in range(NKV):
                        nc.tensor.matmul(
                            ps_v[tt],
                            kvc_sb[k][:, P * tt:P * (tt + 1)],
                            wkv_sb[:, k, :],
                            start=(k == 0), stop=(k == NKV - 1))
                kn_ch = cpool.tile([P, 4, CH], F32R, tag="knch", name="knch",
                                   bufs=2)
                for m in range(4):
                    nc.vector.tensor_tensor(kn_ch[:, m, :], ps_kn[m][:, :CH],
                                            rkv_bc, ALU.mult)
                nc.sync.dma_start(
                    out=bass.AP(tensor=knope_d.tensor, offset=CH * c,
                                ap=[[T, P], [P * T, 4], [1, CH]]),
                    in_=kn_ch)
                v_ch = cpool.tile([P, 2, 512], BF16, tag="vch", name="vch",
                                  bufs=2)
                for tt in range(2):
                    nc.scalar.activation(
                        v_ch[:, tt, :], ps_v[tt], AF.Copy,
                        scale=rkv_tok[:, 2 * c + tt: 2 * c + tt + 1])
                nc.sync.dma_start(
                    out=bass.AP(tensor=v_d.tensor, offset=512 * CH * c,
                                ap=[[512, P], [512 * P, 2], [1, 512]]),
                    in_=v_ch)

            def emit_ag_start(half):
                import os
                cc_out_ap = bass.AP(tensor=cc_out.tensor, offset=half * 4 * HT,
                                    ap=[[HT, 4], [1, HT]])
                if os.environ.get("NO_CC"):
                    for rr in range(4):
                        nc.gpsimd.dma_start(
                            out=bass.AP(tensor=cc_out.tensor,
                                        offset=(half * 4 + rr) * HT,
                                        ap=[[HT, 1], [1, HT]]),
                            in_=cc_in[half:half + 1, :])
                else:
                    nc.gpsimd.collective_compute(
                        "AllGather",
                        ALU.bypass,
                        replica_groups=[[0, 1, 2, 3], [4, 5, 6, 7]],
                        ins=[cc_in[half:half + 1, :]],
                        outs=[cc_out_ap],
                    )

            deferred = {}
            for c in range(NCH):
                kvc_sb, rkv_bc = chunk_sweeps(c)
                if c < DEFER_B:
                    chunk_b(c, kvc_sb, rkv_bc)
                else:
                    deferred[c] = (kvc_sb, rkv_bc)
                if c == 3:
                    emit_ag_start(0)
                if c == 6:
                    emit_ag_finish(0, use_sync_dma=False)
            emit_ag_start(1)
            # deferred B-work lands in the PE queue right where phase II
            # would otherwise starve waiting for q-prep; the pair-0 half-0
            # q prep (loads + DVE) interleaves with it
            cosf = apool.tile([64, T], BF16)
            nc.sync.dma_start(out=cosf, in_=din["cosT4"][0:64, :])
            sinf = apool.tile([64, T], BF16)
            nc.sync.dma_start(out=sinf, in_=din["sinT4"][0:64, :])
            qn_f, qr_f = prep_alloc(qpool, 0)
            prep_pair_half(0, 0, qn_f, qr_f, cosf, sinf)
            for c in sorted(deferred):
                chunk_b(c, *deferred[c])

        # =================== PHASE II: q prep + attention + out ==========
        with ExitStack() as p2:
            qpool2 = p2.enter_context(tc.tile_pool(name="p2q2", bufs=1))
            hpool = p2.enter_context(tc.tile_pool(name="p2h", bufs=2))
            cpool = p2.enter_context(tc.tile_pool(name="p2c", bufs=2))
            espool = p2.enter_context(tc.tile_pool(name="p2e", bufs=2))
            opool = p2.enter_context(tc.tile_pool(name="p2osb", bufs=16))
            oute = p2.enter_context(tc.tile_pool(name="p2oute", bufs=2))
            p2w = p2.enter_context(tc.tile_pool(name="p2w", bufs=1))
            mm2 = p2.enter_context(tc.tile_pool(name="p2s", bufs=3, space="PSUM"))
            omm = p2.enter_context(tc.tile_pool(name="p2o", bufs=2, space="PSUM"))
            p4mm = p2.enter_context(tc.tile_pool(name="p2p4", bufs=3, space="PSUM"))

            def load_head(h, full):
                # uniform tile sizes; block-0 loads fill only the first half
                cols = T if full else HT
                nkt = TT if full else TT // 2
                kn = hpool.tile([P, T], F32R, tag="kn", name=f"kn{h}{full}")
                nc.sync.dma_start(out=kn[:, 0:cols],
                                  in_=knope_d[P * h:P * (h + 1), 0:cols])
                vh = hpool.tile([P, TT, P], BF16, tag="vh", name=f"vh{h}{full}")
                nc.sync.dma_start(
                    out=vh[:, 0:nkt, :],
                    in_=bass.AP(tensor=v_d.tensor, offset=P * h,
                                ap=[[512, P], [P * 512, nkt], [1, P]]))
                return kn, vh

            o_sb = {}

            def emit_p4(qch):
                qsl = slice(512 * qch, 512 * (qch + 1))
                for m in range(DIM // P):
                    ps = p4mm.tile([P, 512], F32, tag="p4", name="p4")
                    for hh in range(4):
                        nc.tensor.matmul(
                            ps, wo_sb[:, hh, P * m:P * (m + 1)],
                            o_sb[(hh, qch)],
                            start=(hh == 0), stop=(hh == 3))
                    ot = oute.tile([P, 512], F32, tag="ot", name="ot")
                    if m % 2 == 0:
                        nc.scalar.activation(ot, ps, AF.Copy)
                    else:
                        nc.vector.tensor_copy(ot, ps)
                    nc.sync.dma_start(out=outT[P * m:P * (m + 1), qsl], in_=ot)

            def attention_qch(h, kn, vh, qn_f, qr_f, qch, with_p4):
                qsl = slice(512 * qch, 512 * (qch + 1))
                n_kt = 4 * (qch + 1)
                es = espool.tile([P, TT, 512], BF16, tag="es", name="es")
                for kt in range(n_kt):
                    ps = mm2.tile([P, 512], F32, tag="s", name="s")
                    nc.tensor.matmul(ps, r32(kn[:, P * kt:P * (kt + 1)]),
                                     r32(qn_f[h][:, qsl]),
                                     start=True, stop=False)
                    nc.tensor.matmul(ps, kpeT[:, P * kt:P * (kt + 1)],
                                     qr_f[h][:, qsl],
                                     start=False, stop=True)
                    di = kt - 4 * qch
                    if di >= 0:
                        nc.vector.tensor_tensor(ps, ps, masks_sb[:, di, :],
                                                ALU.add)
                    nc.scalar.activation(es[:, kt, :], ps, AF.Exp,
                                         scale=SCALE)
                o_ps = omm.tile([P, 512], F32, tag="o", name="o")
                for kt in range(n_kt):
                    nc.tensor.matmul(o_ps, vh[:, kt, :], es[:, kt, :],
                                     start=(kt == 0), stop=(kt == n_kt - 1))
                # softmax denominator off the PE: bf16 tree-sum over the kt
                # tiles, then a cross-partition reduce on GPSIMD
                ses = cpool.tile([P, 512], BF16, tag="ses", name="ses")
                nc.vector.tensor_tensor(ses, es[:, 0, :], es[:, 1, :], ALU.add)
                for kt in range(2, n_kt):
                    nc.vector.tensor_tensor(ses, ses, es[:, kt, :], ALU.add)
                sesum = cpool.tile([P, 512], F32, tag="sesum", name="sesum")
                nc.gpsimd.partition_all_reduce(sesum, ses, channels=P,
                                               reduce_op=bass_isa.ReduceOp.add)
                rec = cpool.tile([P, 512], F32, tag="rec", name="rec")
                nc.vector.reciprocal(rec, sesum)
                och = opool.tile([P, 512], BF16, tag="och", name=f"o{h}{qch}")
                nc.vector.tensor_tensor(och, o_ps, rec, ALU.mult)
                nc.vector.tensor_scalar(och, och, bv_sb[:, h:h + 1], None,
                                        ALU.add)
                o_sb[(h, qch)] = och
                if with_p4:
                    emit_p4(qch)

            # ---- query-block 0 (qch 0,1): needs only rq half 0 ----
            kn0, vh0 = load_head(0, False)
            masks_sb = p2w.tile([P, 4, 512], F32)
            nc.sync.dma_start(
                out=masks_sb,
                in_=bass.AP(tensor=din["masks01"].tensor, offset=0,
                            ap=[[512, P], [P * 512, 4], [1, 512]]))
            attention_qch(0, kn0, vh0, qn_f, qr_f, 0, False)
            attention_qch(0, kn0, vh0, qn_f, qr_f, 1, False)
            qn_f2, qr_f2 = prep_alloc(qpool2, 1)
            prep_pair_half(1, 0, qn_f2, qr_f2, cosf, sinf)
            kn1, vh1 = load_head(1, False)
            attention_qch(1, kn1, vh1, qn_f, qr_f, 0, False)
            attention_qch(1, kn1, vh1, qn_f, qr_f, 1, False)
            wo_sb = p2w.tile([P, 4, T], BF16)
            nc.sync.dma_start(out=wo_sb, in_=_blk(din["woT"], 4, T))
            kn2, vh2 = load_head(2, False)
            attention_qch(2, kn2, vh2, qn_f2, qr_f2, 0, False)
            attention_qch(2, kn2, vh2, qn_f2, qr_f2, 1, False)
            kn3, vh3 = load_head(3, False)
            attention_qch(3, kn3, vh3, qn_f2, qr_f2, 0, True)
            attention_qch(3, kn3, vh3, qn_f2, qr_f2, 1, True)

            # ---- second-half rq: AllGather #2 results, then half-1 q prep;
            # all of this hides under block-0 attention ----
            emit_ag_finish(1, use_sync_dma=False)
            prep_pair_half(0, 1, qn_f, qr_f, cosf, sinf)
            prep_pair_half(1, 1, qn_f2, qr_f2, cosf, sinf)

            # ---- query-block 1 (qch 2,3) ----
            kn0b, vh0b = load_head(0, True)
            attention_qch(0, kn0b, vh0b, qn_f, qr_f, 2, False)
            attention_qch(0, kn0b, vh0b, qn_f, qr_f, 3, False)
            kn1b, vh1b = load_head(1, True)
            attention_qch(1, kn1b, vh1b, qn_f, qr_f, 2, False)
            attention_qch(1, kn1b, vh1b, qn_f, qr_f, 3, False)
            kn2b, vh2b = load_head(2, True)
            attention_qch(2, kn2b, vh2b, qn_f2, qr_f2, 2, False)
            attention_qch(2, kn2b, vh2b, qn_f2, qr_f2, 3, False)
            kn3b, vh3b = load_head(3, True)
            attention_qch(3, kn3b, vh3b, qn_f2, qr_f2, 2, True)
            attention_qch(3, kn3b, vh3b, qn_f2, qr_f2, 3, True)

            if "kpe_o" in scratch:
                nc.sync.dma_start(out=scratch["kpe_o"], in_=kpeT.bitcast(F32))
                nc.sync.dma_start(out=scratch["rq_o"], in_=rq_bc)


# ---------------------------------------------------------------- entry

_NC_CACHE = {}


def _get_nc():
    if "nc" not in _NC_CACHE:
        _NC_CACHE["nc"] = build_bass()
    return _NC_CACHE["nc"]


def _run(inputs, trace=False):
    cores = _host_prep(inputs)
    nc = _get_nc()
    in_maps = [{k: d[k] for k in INPUT_SPECS} for d in cores]
    res = run_bass_kernel_spmd(nc, in_maps, core_ids=list(range(8)), trace=trace)
    outs = [res.results[c]["outT"] for c in range(8)]
    final = np.zeros((B, S, DIM), np.float32)
    wo_b = np.asarray(inputs["wo_b"], np.float32)
    for b in range(B):
        acc = outs[4 * b].copy()
        for g in range(1, HG):
            acc += outs[4 * b + g]
        final[b] = acc.T + wo_b[None, :]
    return final, res


def kernel(**inputs):
    return _run(inputs, trace=False)[0]


def kernel_profiled(**inputs):
    return _run(inputs, trace=False)


# revision 49
# speedup vs baseline: 1.6664x; 1.0645x over previous
"""MLA-style attention (DeepSeek MLA block) on 8 Trainium2 NeuronCores.

Sharding: core c = b*4 + g  (batch b in {0,1}, head-group g in {0..3} = 4 heads).

Key structure (v3):
- The q low-rank path is ABSORBED on the host: Wf = (wq_b * q_norm)[group rows]
  @ wq_a  (768 x 2048). Since the rmsnorm per-token scale r_t commutes through
  the second projection, q = r_t * (Wf x + bf) + b_qb. Each core computes only
  a 384-row slice of wq_a x for the sum-of-squares that defines r_t; the four
  cores of a batch AllGather their 8KB partials (2 collectives, pipelined).
  Collective bounce DMAs must go through gpsimd/SWDGE (HWDGE transfers
  adjacent to a collective get chopped).
- kv path stays two-stage (contraction 512 beats 2048) with kvc replicated.
- k-bias is dropped entirely (softmax is invariant to per-query score shifts);
  v-bias is added after the softmax (rows sum to 1); q biases are applied
  during the on-chip q prep (rope rotation + r_t scaling).
- Inputs x and all first-stage weights are bf16 (same PE throughput as fp32r,
  fp32 accumulation); scores/q/k SBUF tiles stay fp32r. The value path
  (exp(scores), v, attention output, wo) is bf16.
- Softmax denominator via bf16 tree-add + GPSIMD cross-partition reduce
  (keeps the PE free; frees a PSUM bank for score pipelining).
- B-stage for the last chunks is deferred into the phase-II transition to
  keep the PE busy while q-prep loads/rotates.
"""
import ml_dtypes
import numpy as np

import concourse.bass as bass
import concourse.tile as tile
from concourse import bacc, bass_isa, mybir
from concourse.bass_utils import run_bass_kernel_spmd

F32 = mybir.dt.float32
F32R = mybir.dt.float32r
BF16 = mybir.dt.bfloat16
BF_NP = ml_dtypes.bfloat16

B, S, DIM = 2, 2048, 2048
NH = 16
QL, KVL = 1536, 512
NOPE, ROPE, VHD = 128, 64, 128
QK_HD = NOPE + ROPE
EPS = 1e-6
SCALE = QK_HD ** -0.5
HG = 4            # heads per group
T = S
P = 128
KT_D = DIM // P   # 16 contraction tiles over model dim
CH = 256          # phase-I token chunk
NCH = T // CH     # 8 chunks
QSL = QL // HG    # 384 qmid slice rows per core
NKV = 4           # kvc feature tiles (512)
TT = T // P       # 16 token tiles
HT = T // 2
DEFER_B = 4       # chunks >= this get their B-stage after the last sweep


def r32(ap):
    return ap.bitcast(F32R)


# ---------------------------------------------------------------- host side

def _host_prep(inp):
    cos = np.asarray(inp["freqs_cos"], np.float32)   # (S, 32)
    sin = np.asarray(inp["freqs_sin"], np.float32)
    cosT4 = np.ascontiguousarray(np.tile(cos.T, (4, 1))).astype(BF_NP)
    sinT4 = np.ascontiguousarray(np.tile(sin.T, (4, 1))).astype(BF_NP)

    wqa = np.asarray(inp["wq_a_w"], np.float32)          # (1536, 2048)
    bqa = np.asarray(inp["wq_a_b"], np.float32)
    qn = np.asarray(inp["q_norm_w"], np.float32)
    wqb = np.asarray(inp["wq_b_w"], np.float32) * qn[None, :]   # (3072, 1536)
    bqb = np.asarray(inp["wq_b_b"], np.float32)

    perm_kva = np.concatenate([
        np.arange(KVL),
        KVL + 2 * np.arange(32),
        KVL + 2 * np.arange(32) + 1,
    ])
    wkvaT = np.ascontiguousarray(
        np.asarray(inp["wkv_a_w"], np.float32)[perm_kva].T).astype(BF_NP)
    bkva_p = np.asarray(inp["wkv_a_b"], np.float32)[perm_kva]
    bkva = np.zeros((5, P), np.float32)
    bkva.reshape(-1)[:576] = bkva_p

    kvn = np.asarray(inp["kv_norm_w"], np.float32)
    wkvb = np.asarray(inp["wkv_b_w"], np.float32) * kvn[None, :]
    bkvb = np.asarray(inp["wkv_b_b"], np.float32)
    wo = np.asarray(inp["wo_w"], np.float32)

    # additive causal masks per diagonal sub-tile: -1e30 where (128*di+k) > q
    masks01 = np.zeros((4, P, 512), np.float32)
    for di in range(4):
        kk = np.arange(P)[:, None] + P * di
        qq = np.arange(512)[None, :]
        masks01[di] = np.where(kk > qq, -1e30, 0.0).astype(np.float32)

    cores = []
    for b in range(B):
        xt = np.ascontiguousarray(
            np.asarray(inp["x"], np.float32)[b].T).astype(BF_NP)
        for g in range(HG):
            heads = range(4 * g, 4 * g + 4)
            rows_nope = np.concatenate(
                [np.arange(h * QK_HD, h * QK_HD + NOPE) for h in heads])
            rows_real = np.concatenate(
                [h * QK_HD + NOPE + 2 * np.arange(32) for h in heads])
            rows_imag = np.concatenate(
                [h * QK_HD + NOPE + 2 * np.arange(32) + 1 for h in heads])
            rows_q = np.concatenate([rows_nope, rows_real, rows_imag])

            wf = wqb[rows_q] @ wqa                     # (768, 2048)
            bf = wqb[rows_q] @ bqa                     # (768,)
            bqb_n = bqb[rows_nope]                     # (512,)
            br, bi = bqb[rows_real], bqb[rows_imag]    # (128,) each
            fidx = np.tile(np.arange(32), 4)
            broped_r = (br[:, None] * cos.T[fidx] - bi[:, None] * sin.T[fidx])
            broped_i = (br[:, None] * sin.T[fidx] + bi[:, None] * cos.T[fidx])

            rows_k = np.concatenate(
                [np.arange(h * (NOPE + VHD), h * (NOPE + VHD) + NOPE) for h in heads])
            rows_v = np.concatenate(
                [np.arange(h * (NOPE + VHD) + NOPE, (h + 1) * (NOPE + VHD)) for h in heads])

            # packed per-feature scalars, cols: 0:3 bqa_slice | 3:8 bkva |
            # 8:14 bfused | 14:18 bqb_nope | 18:22 bv | 22 eps
            biases = np.zeros((32, P), np.float32)
            biases[0:3] = bqa[QSL * g: QSL * (g + 1)].reshape(3, P)
            biases[3:8] = bkva
            biases[8:14] = bf.reshape(6, P)
            biases[14:18] = bqb_n.reshape(4, P)
            biases[18:22] = bkvb[rows_v].reshape(4, P)
            biases[22] = EPS
            cores.append(dict(
                xt=xt,
                wqaT_sl=np.ascontiguousarray(
                    wqa[QSL * g: QSL * (g + 1)].T).astype(BF_NP),
                wkvaT=wkvaT,
                wfT=np.ascontiguousarray(wf.T).astype(BF_NP),
                wkvbTk=np.ascontiguousarray(wkvb[rows_k].T).astype(BF_NP),
                wkvbTv=np.ascontiguousarray(wkvb[rows_v].T).astype(BF_NP),
                woT=np.ascontiguousarray(
                    wo[:, 512 * g: 512 * (g + 1)].T).astype(BF_NP),
                biases=biases,
                cosT4=cosT4, sinT4=sinT4,
                broped_r=np.ascontiguousarray(broped_r).astype(BF_NP),
                broped_i=np.ascontiguousarray(broped_i).astype(BF_NP),
                masks01=masks01,
                ones_bf=np.ones((P, 1), BF_NP),
            ))
    return cores


INPUT_SPECS = dict(
    xt=((DIM, T), BF16),
    wqaT_sl=((DIM, QSL), BF16),
    wkvaT=((DIM, 576), BF16),
    wfT=((DIM, 768), BF16),
    wkvbTk=((KVL, 512), BF16),
    wkvbTv=((KVL, 512), BF16),
    woT=((512, DIM), BF16),
    ones_bf=((P, 1), BF16),
    biases=((32, P), F32),
    cosT4=((P, T), BF16), sinT4=((P, T), BF16),
    broped_r=((P, T), BF16), broped_i=((P, T), BF16),
    masks01=((4, P, 512), F32),
)


# ---------------------------------------------------------------- device IR

def _blk(w, kt, width, m0=0):
    """AP over DRAM weight w (R, C): (128p rows, kt ktiles, width cols at m0)."""
    rows, cols = w.shape
    return bass.AP(
        tensor=w.tensor, offset=m0,
        ap=[[cols, P], [P * cols, kt], [1, width]],
    )


def _t_view(a2d):
    arows, acols = a2d.shape
    assert acols == P
    return bass.AP(tensor=a2d.tensor, offset=0, ap=[[1, P], [P, arows]])


def build_bass(debug_taps=False):
    nc = bacc.Bacc("TRN2", target_bir_lowering=False, debug=False, num_devices=8)

    din = {name: nc.dram_tensor(name, shape, dt, kind="ExternalInput").ap()
           for name, (shape, dt) in INPUT_SPECS.items()}
    outT = nc.dram_tensor("outT", (DIM, T), BF16, kind="ExternalOutput").ap()
    kind = "ExternalOutput" if debug_taps else "Internal"
    scratch = dict(
        qT_d=nc.dram_tensor("qT_d", (768, T), BF16, kind=kind).ap(),
        knope_d=nc.dram_tensor("knope_d", (512, T), F32R, kind=kind).ap(),
        v_d=nc.dram_tensor("v_d", (T, 512), BF16, kind=kind).ap(),
        cc_in=nc.dram_tensor("cc_in", (2, HT), F32).ap(),
        cc_out=nc.dram_tensor("cc_out", (2, 4, HT), F32).ap(),
    )
    if debug_taps:
        scratch["kpe_o"] = nc.dram_tensor("kpe_o", (64, T), F32, kind=kind).ap()
        scratch["rq_o"] = nc.dram_tensor("rq_o", (P, T), F32, kind=kind).ap()

    with tile.TileContext(nc) as tc:
        _emit(tc, din, outT, scratch)

    nc.compile()
    return nc


def _emit(tc, din, outT, scratch):
    nc = tc.nc
    from contextlib import ExitStack
    ALU = mybir.AluOpType
    AF = mybir.ActivationFunctionType
    qT_d, knope_d, v_d = scratch["qT_d"], scratch["knope_d"], scratch["v_d"]
    cc_in, cc_out = scratch["cc_in"], scratch["cc_out"]

    with ExitStack() as outer:
        const = outer.enter_context(tc.tile_pool(name="const", bufs=1))
        ones_bf = const.tile([P, 1], BF16)
        nc.sync.dma_start(out=ones_bf, in_=din["ones_bf"])
        bs = const.tile([P, 32], F32)
        nc.sync.dma_start(out=bs, in_=_t_view(din["biases"]))
        bqa_sb, bkva_sb = bs[:, 0:3], bs[:, 3:8]
        bf_sb, bqbn_sb, bv_sb = bs[:, 8:14], bs[:, 14:18], bs[:, 18:22]
        eps_sb = bs[:, 22:23]
        kpeT = const.tile([64, T], BF16)      # roped shared k_pe [real|imag]
        rkv_tok = const.tile([P, TT], F32)
        rq_bc = const.tile([P, T], F32)       # broadcast 1/rms(q) per token
        apool = outer.enter_context(tc.tile_pool(name="p2a", bufs=1))
        qpool = outer.enter_context(tc.tile_pool(name="p2q", bufs=1))
        prep = outer.enter_context(tc.tile_pool(name="p2p", bufs=1))
        agp = outer.enter_context(tc.tile_pool(name="agp", bufs=1))
        cc_out_t = scratch["cc_out"]

        def emit_ag_finish(half, use_sync_dma):
            AF = mybir.ActivationFunctionType
            for qc in range(4):
                off = half * 4 * HT + CH * qc
                gath = agp.tile([4, CH], F32, tag="gath", name="gath", bufs=1)
                dma = nc.sync.dma_start if use_sync_dma else nc.gpsimd.dma_start
                dma(out=gath,
                    in_=bass.AP(tensor=cc_out_t.tensor, offset=off,
                                ap=[[HT, 4], [1, CH]]))
                asum = agp.tile([4, CH], F32, tag="asum", name="asum", bufs=1)
                nc.gpsimd.partition_all_reduce(
                    asum, gath, channels=4, reduce_op=bass_isa.ReduceOp.add)
                srt = agp.tile([1, CH], F32, tag="qsrt", name="qsrt", bufs=1)
                nc.scalar.activation(srt, asum[0:1, :], AF.Sqrt,
                                     bias=eps_sb[0:1, :], scale=1.0 / QL)
                rqh = agp.tile([1, CH], F32, tag="rqh", name="rqh", bufs=1)
                nc.vector.reciprocal(rqh, srt)
                gsl = slice(HT * half + CH * qc, HT * half + CH * (qc + 1))
                nc.gpsimd.partition_broadcast(rq_bc[:, gsl], rqh)

        def prep_alloc(pool, pair):
            qn_f, qr_f = {}, {}
            for i in range(2):
                h = 2 * pair + i
                qn_f[h] = pool.tile([P, T], F32R, tag=f"qn{i}", name=f"qnf{h}")
                qr_f[h] = pool.tile([64, T], BF16, tag=f"qr{i}", name=f"qrf{h}")
            return qn_f, qr_f

        def prep_pair_half(pair, half, qn_f, qr_f, cosf, sinf):
            ALU = mybir.AluOpType
            qT_d = scratch["qT_d"]
            hsl = slice(HT * half, HT * (half + 1))
            for i in range(2):
                h = 2 * pair + i
                tmp = prep.tile([P, HT], BF16, tag="qtmp", name="qtmp")
                nc.sync.dma_start(out=tmp, in_=qT_d[P * h:P * (h + 1), hsl])
                nc.vector.tensor_tensor(qn_f[h][:, hsl], tmp,
                                        rq_bc[:, hsl], ALU.mult)
                nc.vector.tensor_scalar(
                    qn_f[h][:, hsl], qn_f[h][:, hsl],
                    bqbn_sb[:, h:h + 1], None, ALU.add)
            xr = prep.tile([64, HT], BF16, tag="xr", name="xr")
            nc.sync.dma_start(
                out=xr, in_=qT_d[512 + 64 * pair:512 + 64 * pair + 64, hsl])
            xi = prep.tile([64, HT], BF16, tag="xi", name="xi")
            nc.sync.dma_start(
                out=xi, in_=qT_d[640 + 64 * pair:640 + 64 * pair + 64, hsl])
            bro_r = prep.tile([64, HT], BF16, tag="bror", name="bror")
            nc.sync.dma_start(
                out=bro_r, in_=din["broped_r"][64 * pair:64 * pair + 64, hsl])
            bro_i = prep.tile([64, HT], BF16, tag="broi", name="broi")
            nc.sync.dma_start(
                out=bro_i, in_=din["broped_i"][64 * pair:64 * pair + 64, hsl])
            c_, s_ = cosf[:, hsl], sinf[:, hsl]
            t1 = prep.tile([64, HT], BF16, tag="t1", name="t1")
            t2 = prep.tile([64, HT], BF16, tag="t2", name="t2")
            yr = prep.tile([64, HT], BF16, tag="yr", name="yr")
            yi = prep.tile([64, HT], BF16, tag="yi", name="yi")
            nc.vector.tensor_tensor(t1, xr, c_, ALU.mult)
            nc.vector.tensor_tensor(t2, xi, s_, ALU.mult)
            nc.vector.tensor_tensor(yr, t1, t2, ALU.subtract)
            nc.vector.tensor_tensor(t1, xr, s_, ALU.mult)
            nc.vector.tensor_tensor(t2, xi, c_, ALU.mult)
            nc.vector.tensor_tensor(yi, t1, t2, ALU.add)
            nc.vector.tensor_tensor(yr, yr, rq_bc[0:64, hsl], ALU.mult)
            nc.vector.tensor_tensor(yr, yr, bro_r, ALU.add)
            nc.vector.tensor_tensor(yi, yi, rq_bc[0:64, hsl], ALU.mult)
            nc.vector.tensor_tensor(yi, yi, bro_i, ALU.add)
            for i in range(2):
                h = 2 * pair + i
                nc.sync.dma_start(out=qr_f[h][0:32, hsl],
                                  in_=yr[32 * i:32 * i + 32, :])
                nc.sync.dma_start(out=qr_f[h][32:64, hsl],
                                  in_=yi[32 * i:32 * i + 32, :])

        # =================== PHASE I: projections ======================
        with ExitStack() as p1:
            wpool = p1.enter_context(tc.tile_pool(name="w1", bufs=1))
            xpool = p1.enter_context(tc.tile_pool(name="p1x", bufs=2))

            # weights in consumption order, block-granular so each chain
            # starts as its block lands; x0 split so the first chain can
            # begin after ~1.2MB of DMA
            wqa_sb = [wpool.tile([P, KT_D, P], BF16, tag=f"wqa{m}",
                                 name=f"wqa{m}") for m in range(3)]
            nc.sync.dma_start(out=wqa_sb[0], in_=_blk(din["wqaT_sl"], KT_D, P, 0))
            x0_sb = xpool.tile([P, KT_D, CH], BF16, tag="x", name="x0")
            nc.sync.dma_start(
                out=x0_sb[:, 0:8, :],
                in_=bass.AP(tensor=din["xt"].tensor, offset=0,
                            ap=[[T, P], [P * T, 8], [1, CH]]))
            nc.sync.dma_start(
                out=x0_sb[:, 8:KT_D, :],
                in_=bass.AP(tensor=din["xt"].tensor, offset=8 * P * T,
                            ap=[[T, P], [P * T, 8], [1, CH]]))
            for m in range(1, 3):
                nc.sync.dma_start(out=wqa_sb[m],
                                  in_=_blk(din["wqaT_sl"], KT_D, P, P * m))
            wkva_sb = []
            for m in range(5):
                w = 64 if m == 4 else P
                t = wpool.tile([P, KT_D, w], BF16, tag=f"wkva{m}", name=f"wkva{m}")
                nc.sync.dma_start(out=t, in_=_blk(din["wkvaT"], KT_D, w, P * m))
                wkva_sb.append(t)
            wf_sb = []
            for m in range(6):
                t = wpool.tile([P, KT_D, P], BF16, tag=f"wf{m}", name=f"wf{m}")
                nc.sync.dma_start(out=t, in_=_blk(din["wfT"], KT_D, P, P * m))
                wf_sb.append(t)
            wkk_sb = wpool.tile([P, NKV, 512], BF16)
            nc.sync.dma_start(out=wkk_sb, in_=_blk(din["wkvbTk"], NKV, 512))
            wkv_sb = wpool.tile([P, NKV, 512], BF16)
            nc.sync.dma_start(out=wkv_sb, in_=_blk(din["wkvbTv"], NKV, 512))

            cpool = p1.enter_context(tc.tile_pool(name="p1c", bufs=2))
            kvpool = p1.enter_context(tc.tile_pool(name="p1kv", bufs=1))
            mm = p1.enter_context(tc.tile_pool(name="p1ps", bufs=6, space="PSUM"))
            sspool = p1.enter_context(tc.tile_pool(name="p1ss", bufs=2, space="PSUM"))

            x_tiles = {0: x0_sb}

            def prefetch_x(c):
                if c < NCH and c not in x_tiles:
                    t = xpool.tile([P, KT_D, CH], BF16, tag="x", name=f"x{c}")
                    nc.sync.dma_start(
                        out=t,
                        in_=bass.AP(tensor=din["xt"].tensor, offset=CH * c,
                                    ap=[[T, P], [P * T, KT_D], [1, CH]]))
                    x_tiles[c] = t

            def chunk_sweeps(c):
                csl = slice(CH * c, CH * (c + 1))
                prefetch_x(c)
                x_sb = x_tiles.pop(c)

                # ---- sweep 1: qmid-slice (3) + kvc (4) + kpe (1) chains ----
                ps_q = [mm.tile([P, 512], F32, tag="mm", name=f"q{m}")
                        for m in range(3)]
                for m in range(3):
                    for k in range(KT_D):
                        nc.tensor.matmul(
                            ps_q[m][:, :CH],
                            wqa_sb[m][:, k, :],
                            x_sb[:, k, :],
                            start=(k == 0), stop=(k == KT_D - 1))
                ps_kv = [mm.tile([P, 512], F32, tag="mm", name=f"kv{m}")
                         for m in range(4)]
                for m in range(4):
                    for k in range(KT_D):
                        nc.tensor.matmul(
                            ps_kv[m][:, :CH],
                            wkva_sb[m][:, k, :],
                            x_sb[:, k, :],
                            start=(k == 0), stop=(k == KT_D - 1))
                ps_kpe = mm.tile([P, 512], F32, tag="mm", name="kpe")
                for k in range(KT_D):
                    nc.tensor.matmul(
                        ps_kpe[:64, :CH],
                        wkva_sb[4][:, k, :],
                        x_sb[:, k, :],
                        start=(k == 0), stop=(k == KT_D - 1))

                prefetch_x(c + 1)

                # ---- sweep-1 evictions ----
                sq = []   # squared qmid slices (bf16: feeds a bf16 sum-MM)
                for m in range(3):
                    t = cpool.tile([P, CH], BF16, tag=f"sq{m}", name=f"sq{m}", bufs=1)
                    nc.scalar.activation(t, ps_q[m][:, :CH], AF.Square,
                                         bias=bqa_sb[:, m:m + 1])
                    sq.append(t)
                kvc_sb, kvsq = [], []
                for m in range(4):
                    t = kvpool.tile([P, CH], BF16, tag=f"kvc{m}", name=f"kvc{m}",
                                    bufs=4)
                    nc.vector.tensor_scalar(t, ps_kv[m][:, :CH],
                                            bkva_sb[:, m:m + 1], None, ALU.add)
                    kvc_sb.append(t)
                    t2 = cpool.tile([P, CH], BF16, tag=f"kvsq{m}", name=f"kvsq{m}",
                                    bufs=1)
                    nc.scalar.activation(t2, ps_kv[m][:, :CH], AF.Square,
                                         bias=bkva_sb[:, m:m + 1])
                    kvsq.append(t2)
                kpe_raw = cpool.tile([64, CH], F32, tag="kpr", name="kpr", bufs=1)
                nc.scalar.activation(kpe_raw, ps_kpe[:64, :CH], AF.Identity,
                                     bias=bkva_sb[0:64, 4:5])
                # rope-rotate k_pe into kpeT[:, csl]; imag half of the input
                # is DMA-shifted down to partitions 0..31 first
                c32 = cpool.tile([32, CH], BF16, tag="c32", name="c32", bufs=1)
                nc.sync.dma_start(out=c32, in_=din["cosT4"][0:32, csl])
                s32 = cpool.tile([32, CH], BF16, tag="s32", name="s32", bufs=1)
                nc.sync.dma_start(out=s32, in_=din["sinT4"][0:32, csl])
                xr = kpe_raw[0:32, :]
                xi = cpool.tile([32, CH], F32, tag="xikp", name="xikp", bufs=1)
                nc.sync.dma_start(out=xi, in_=kpe_raw[32:64, :])
                t2_ = cpool.tile([32, CH], F32, tag="t2", name="t2", bufs=1)
                yikp = cpool.tile([32, CH], BF16, tag="yikp", name="yikp", bufs=1)
                nc.vector.tensor_tensor(kpeT[0:32, csl], xr, c32, ALU.mult)
                nc.vector.tensor_tensor(t2_, xi, s32, ALU.mult)
                nc.vector.tensor_tensor(kpeT[0:32, csl], kpeT[0:32, csl], t2_,
                                        ALU.subtract)
                nc.vector.tensor_tensor(yikp, xr, s32, ALU.mult)
                nc.vector.tensor_tensor(t2_, xi, c32, ALU.mult)
                nc.vector.tensor_tensor(yikp, yikp, t2_, ALU.add)
                nc.sync.dma_start(out=kpeT[32:64, csl], in_=yikp)

                # ---- sweep 2: fused-q (6 chains) + sumsq chains ----
                ps_fq = [mm.tile([P, 512], F32, tag="mm", name=f"fq{m}")
                         for m in range(6)]
                for m in range(6):
                    for k in range(KT_D):
                        nc.tensor.matmul(
                            ps_fq[m][:, :CH],
                            wf_sb[m][:, k, :],
                            x_sb[:, k, :],
                            start=(k == 0), stop=(k == KT_D - 1))
                ps_qss = sspool.tile([1, 512], F32, tag="ss", name="qss")
                for m in range(3):
                    nc.tensor.matmul(ps_qss[:, :CH], ones_bf, sq[m],
                                     start=(m == 0), stop=(m == 2))
                ps_kss = sspool.tile([1, 512], F32, tag="ss", name="kss")
                for m in range(4):
                    nc.tensor.matmul(ps_kss[:, :CH], ones_bf, kvsq[m],
                                     start=(m == 0), stop=(m == 3))

                # ---- sweep-2 evictions ----
                qch = cpool.tile([P, 6, CH], BF16, tag="qch", name="qch", bufs=1)
                for m in range(6):
                    nc.scalar.activation(qch[:, m, :], ps_fq[m][:, :CH],
                                         AF.Identity, bias=bf_sb[:, m:m + 1])
                nc.sync.dma_start(
                    out=bass.AP(tensor=qT_d.tensor, offset=CH * c,
                                ap=[[T, P], [P * T, 6], [1, CH]]),
                    in_=qch)
                # q-sumsq partial straight to the collective input buffer
                qssv = cpool.tile([1, CH], F32, tag="qssv", name="qssv", bufs=1)
                nc.vector.tensor_copy(qssv, ps_qss[:, :CH])
                nc.gpsimd.dma_start(
                    out=bass.AP(tensor=cc_in.tensor, offset=CH * c,
                                ap=[[CH, 1], [1, CH]]),
                    in_=qssv)
                # rkv for this chunk
                srt = cpool.tile([1, CH], F32, tag="srt", name="srt", bufs=1)
                nc.scalar.activation(srt, ps_kss[:, :CH], AF.Sqrt,
                                     bias=eps_sb[0:1, :], scale=1.0 / KVL)
                rkv_c = cpool.tile([1, CH], F32, tag="rkvc", name="rkvc", bufs=1)
                nc.vector.reciprocal(rkv_c, srt)
                rkv_bc = cpool.tile([P, CH], F32, tag="rkvbc", name="rkvbc",
                                    bufs=4)
                nc.gpsimd.partition_broadcast(rkv_bc, rkv_c)
                for tt in range(2):
                    nc.sync.dma_start(
                        out=rkv_tok[:, 2 * c + tt: 2 * c + tt + 1],
                        in_=rkv_c[:, P * tt:P * (tt + 1)])
                return kvc_sb, rkv_bc

            def chunk_b(c, kvc_sb, rkv_bc):
                ps_kn = [mm.tile([P, 512], F32, tag="mm", name=f"kn{m}")
                         for m in range(4)]
                for m in range(4):
                    for k in range(NKV):
                        nc.tensor.matmul(
                            ps_kn[m][:, :CH],
                            wkk_sb[:, k, P * m:P * (m + 1)],
                            kvc_sb[k],
                            start=(k == 0), stop=(k == NKV - 1))
                ps_v = [mm.tile([P, 512], F32, tag="mm", name=f"v{tt}")
                        for tt in range(2)]
                for tt in range(2):
                    for k in range(NKV):
                        nc.tensor.matmul(
                            ps_v[tt],
                            kvc_sb[k][:, P * tt:P * (tt + 1)],
                            wkv_sb[:, k, :],
                            start=(k == 0), stop=(k == NKV - 1))
                kn_ch = cpool.tile([P, 4, CH], F32R, tag="knch", name="knch",
                                   bufs=2)
                for m in range(4):
                    nc.vector.tensor_tensor(kn_ch[:, m, :], ps_kn[m][:, :CH],
                                            rkv_bc, ALU.mult)
                nc.sync.dma_start(
                    out=bass.AP(tensor=knope_d.tensor, offset=CH * c,
                                ap=[[T, P], [P * T, 4], [1, CH]]),
                    in_=kn_ch)
                v_ch = cpool.tile([P, 2, 512], BF16, tag="vch", name="vch",
                                  bufs=2)
                for tt in range(2):
                    nc.scalar.activation(
                        v_ch[:, tt, :], ps_v[tt], AF.Copy,
                        scale=rkv_tok[:, 2 * c + tt: 2 * c + tt + 1])
                nc.sync.dma_start(
                    out=bass.AP(tensor=v_d.tensor, offset=512 * CH * c,
                                ap=[[512, P], [512 * P, 2], [1, 512]]),
                    in_=v_ch)

            def emit_ag_start(half):
                import os
                cc_out_ap = bass.AP(tensor=cc_out.tensor, offset=half * 4 * HT,
                                    ap=[[HT, 4], [1, HT]])
                if os.environ.get("NO_CC"):
                    for rr in range(4):
                        nc.gpsimd.dma_start(
                            out=bass.AP(tensor=cc_out.tensor,
                                        offset=(half * 4 + rr) * HT,
                                        ap=[[HT, 1], [1, HT]]),
                            in_=cc_in[half:half + 1, :])
                else:
                    nc.gpsimd.collective_compute(
                        "AllGather",
                        ALU.bypass,
                        replica_groups=[[0, 1, 2, 3], [4, 5, 6, 7]],
                        ins=[cc_in[half:half + 1, :]],
                        outs=[cc_out_ap],
                    )

            masks_sb = apool.tile([P, 4, 512], F32)
            deferred = {}
            for c in range(NCH):
                if c == 2:
                    nc.sync.dma_start(
                        out=masks_sb,
                        in_=bass.AP(tensor=din["masks01"].tensor, offset=0,
                                    ap=[[512, P], [P * 512, 4], [1, 512]]))
                kvc_sb, rkv_bc = chunk_sweeps(c)
                if c < DEFER_B:
                    chunk_b(c, kvc_sb, rkv_bc)
                else:
                    deferred[c] = (kvc_sb, rkv_bc)
                if c == 3:
                    emit_ag_start(0)
                if c == 6:
                    emit_ag_finish(0, use_sync_dma=False)
            emit_ag_start(1)
            # deferred B-work lands in the PE queue right where phase II
            # would otherwise starve waiting for q-prep; the pair-0 half-0
            # q prep (loads + DVE) interleaves with it
            cosf = apool.tile([64, T], BF16)
            nc.sync.dma_start(out=cosf, in_=din["cosT4"][0:64, :])
            sinf = apool.tile([64, T], BF16)
            nc.sync.dma_start(out=sinf, in_=din["sinT4"][0:64, :])
            qn_f, qr_f = prep_alloc(qpool, 0)
            prep_pair_half(0, 0, qn_f, qr_f, cosf, sinf)
            for c in sorted(deferred):
                chunk_b(c, *deferred[c])

        # =================== PHASE II: q prep + attention + out ==========
        with ExitStack() as p2:
            qpool2 = p2.enter_context(tc.tile_pool(name="p2q2", bufs=1))
            hpool = p2.enter_context(tc.tile_pool(name="p2h", bufs=2))
            cpool = p2.enter_context(tc.tile_pool(name="p2c", bufs=2))
            espool = p2.enter_context(tc.tile_pool(name="p2e", bufs=2))
            opool = p2.enter_context(tc.tile_pool(name="p2osb", bufs=16))
            oute = p2.enter_context(tc.tile_pool(name="p2oute", bufs=2))
            p2w = p2.enter_context(tc.tile_pool(name="p2w", bufs=1))
            mm2 = p2.enter_context(tc.tile_pool(name="p2s", bufs=3, space="PSUM"))
            omm = p2.enter_context(tc.tile_pool(name="p2o", bufs=2, space="PSUM"))
            p4mm = p2.enter_context(tc.tile_pool(name="p2p4", bufs=3, space="PSUM"))

            def load_head(h, full):
                # uniform tile sizes; block-0 loads fill only the first half
                cols = T if full else HT
                nkt = TT if full else TT // 2
                kn = hpool.tile([P, T], F32R, tag="kn", name=f"kn{h}{full}")
                nc.sync.dma_start(out=kn[:, 0:cols],
                                  in_=knope_d[P * h:P * (h + 1), 0:cols])
                vh = hpool.tile([P, TT, P], BF16, tag="vh", name=f"vh{h}{full}")
                nc.sync.dma_start(
                    out=vh[:, 0:nkt, :],
                    in_=bass.AP(tensor=v_d.tensor, offset=P * h,
                                ap=[[512, P], [P * 512, nkt], [1, P]]))
                return kn, vh

            o_sb = {}

            def emit_p4(qch):
                qsl = slice(512 * qch, 512 * (qch + 1))
                for m in range(DIM // P):
                    ps = p4mm.tile([P, 512], F32, tag="p4", name="p4")
                    for hh in range(4):
                        nc.tensor.matmul(
                            ps, wo_sb[:, hh, P * m:P * (m + 1)],
                            o_sb[(hh, qch)],
                            start=(hh == 0), stop=(hh == 3))
                    ot = oute.tile([P, 512], BF16, tag="ot", name="ot")
                    if m % 2 == 0:
                        nc.scalar.activation(ot, ps, AF.Copy)
                    else:
                        nc.vector.tensor_copy(ot, ps)
                    nc.sync.dma_start(out=outT[P * m:P * (m + 1), qsl], in_=ot)

            def attention_qch(h, kn, vh, qn_f, qr_f, qch, with_p4):
                q0 = 512 * qch
                n_kt = 4 * (qch + 1)
                # per-kt valid query range: diagonal sub-tiles only cover
                # queries >= 128*di within the block (causality)
                lo = [max(0, 128 * (kt - 4 * qch)) for kt in range(n_kt)]
                es = espool.tile([P, TT, 512], BF16, tag="es", name="es")
                for kt in range(n_kt):
                    l = lo[kt]
                    # fp32r needs free >= 256 for full speed; keep the nope
                    # matmul full-width when the trimmed width drops below
                    nl = l if 512 - l >= 256 else 0
                    ps = mm2.tile([P, 512], F32, tag="s", name="s")
                    nc.tensor.matmul(ps[:, nl:512],
                                     r32(kn[:, P * kt:P * (kt + 1)]),
                                     r32(qn_f[h][:, q0 + nl:q0 + 512]),
                                     start=True, stop=False)
                    nc.tensor.matmul(ps[:, l:512], kpeT[:, P * kt:P * (kt + 1)],
                                     qr_f[h][:, q0 + l:q0 + 512],
                                     start=False, stop=True)
                    if kt - 4 * qch >= 0:
                        di = kt - 4 * qch
                        nc.vector.tensor_tensor(
                            ps[:, l:l + P], ps[:, l:l + P],
                            masks_sb[:, di, l:l + P], ALU.add)
                    nc.scalar.activation(es[:, kt, l:512], ps[:, l:512],
                                         AF.Exp, scale=SCALE)
                o_ps = omm.tile([P, 512], F32, tag="o", name="o")
                for kt in range(n_kt):
                    l = lo[kt]
                    nc.tensor.matmul(o_ps[:, l:512], vh[:, kt, :],
                                     es[:, kt, l:512],
                                     start=(kt == 0), stop=(kt == n_kt - 1))
                # softmax denominator off the PE: bf16 tree-sum over the kt
                # tiles, then a cross-partition reduce on GPSIMD
                ses = cpool.tile([P, 512], BF16, tag="ses", name="ses")
                nc.vector.tensor_copy(ses, es[:, 0, :])
                for kt in range(1, n_kt):
                    l = lo[kt]
                    nc.vector.tensor_tensor(ses[:, l:512], ses[:, l:512],
                                            es[:, kt, l:512], ALU.add)
                sesum = cpool.tile([P, 512], F32, tag="sesum", name="sesum")
                nc.gpsimd.partition_all_reduce(sesum, ses, channels=P,
                                               reduce_op=bass_isa.ReduceOp.add)
                rec = cpool.tile([P, 512], F32, tag="rec", name="rec")
                nc.vector.reciprocal(rec, sesum)
                och = opool.tile([P, 512], BF16, tag="och", name=f"o{h}{qch}")
                nc.vector.tensor_tensor(och, o_ps, rec, ALU.mult)
                nc.vector.tensor_scalar(och, och, bv_sb[:, h:h + 1], None,
                                        ALU.add)
                o_sb[(h, qch)] = och
                if with_p4:
                    emit_p4(qch)

            # ---- query-block 0 (qch 0,1): needs only rq half 0 ----
            kn0, vh0 = load_head(0, False)
            attention_qch(0, kn0, vh0, qn_f, qr_f, 0, False)
            attention_qch(0, kn0, vh0, qn_f, qr_f, 1, False)
            qn_f2, qr_f2 = prep_alloc(qpool2, 1)
            prep_pair_half(1, 0, qn_f2, qr_f2, cosf, sinf)
            kn1, vh1 = load_head(1, False)
            attention_qch(1, kn1, vh1, qn_f, qr_f, 0, False)
            attention_qch(1, kn1, vh1, qn_f, qr_f, 1, False)
            kn2, vh2 = load_head(2, False)
            attention_qch(2, kn2, vh2, qn_f2, qr_f2, 0, False)
            attention_qch(2, kn2, vh2, qn_f2, qr_f2, 1, False)
            kn3, vh3 = load_head(3, False)
            wo_sb = p2w.tile([P, 4, T], BF16)
            nc.sync.dma_start(out=wo_sb, in_=_blk(din["woT"], 4, T))
            attention_qch(3, kn3, vh3, qn_f2, qr_f2, 0, True)
            attention_qch(3, kn3, vh3, qn_f2, qr_f2, 1, True)

            # ---- second-half rq: AllGather #2 results, then half-1 q prep;
            # all of this hides under block-0 attention ----
            emit_ag_finish(1, use_sync_dma=False)
            prep_pair_half(0, 1, qn_f, qr_f, cosf, sinf)
            prep_pair_half(1, 1, qn_f2, qr_f2, cosf, sinf)

            # ---- query-block 1 (qch 2,3) ----
            kn0b, vh0b = load_head(0, True)
            attention_qch(0, kn0b, vh0b, qn_f, qr_f, 2, False)
            attention_qch(0, kn0b, vh0b, qn_f, qr_f, 3, False)
            kn1b, vh1b = load_head(1, True)
            attention_qch(1, kn1b, vh1b, qn_f, qr_f, 2, False)
            attention_qch(1, kn1b, vh1b, qn_f, qr_f, 3, False)
            kn2b, vh2b = load_head(2, True)
            attention_qch(2, kn2b, vh2b, qn_f2, qr_f2, 2, False)
            attention_qch(2, kn2b, vh2b, qn_f2, qr_f2, 3, False)
            kn3b, vh3b = load_head(3, True)
            attention_qch(3, kn3b, vh3b, qn_f2, qr_f2, 2, True)
            attention_qch(3, kn3b, vh3b, qn_f2, qr_f2, 3, True)

            if "kpe_o" in scratch:
                nc.sync.dma_start(out=scratch["kpe_o"], in_=kpeT.bitcast(F32))
                nc.sync.dma_start(out=scratch["rq_o"], in_=rq_bc)


# ---------------------------------------------------------------- entry

_NC_CACHE = {}


def _get_nc():
    if "nc" not in _NC_CACHE:
        _NC_CACHE["nc"] = build_bass()
    return _NC_CACHE["nc"]


def _run(inputs, trace=False):
    cores = _host_prep(inputs)
    nc = _get_nc()
    in_maps = [{k: d[k] for k in INPUT_SPECS} for d in cores]
    res = run_bass_kernel_spmd(nc, in_maps, core_ids=list(range(8)), trace=trace)
    outs = [np.asarray(res.results[c]["outT"], np.float32) for c in range(8)]
    final = np.zeros((B, S, DIM), np.float32)
    wo_b = np.asarray(inputs["wo_b"], np.float32)
    for b in range(B):
        acc = outs[4 * b].copy()
        for g in range(1, HG):
            acc += outs[4 * b + g]
        final[b] = acc.T + wo_b[None, :]
    return final, res


def kernel(**inputs):
    return _run(inputs, trace=False)[0]


def kernel_profiled(**inputs):
    return _run(inputs, trace=False)
